# revision 25
# baseline (speedup 1.0000x reference)
"""Differential multi-head attention on 8 Trainium2 NeuronCores.

Sharding: tensor-parallel over heads x data-parallel over batch.
Core c handles batch b = c//4 and real heads [4*(c%4), 4*(c%4)+4).
Each core computes a partial output (its 256 attention features through
the output projection); the host sums the 4 partials per batch.

Per-core design (v2) -- three-engine exp + fp8 DoubleRow matmuls:

  The kernel is exp-bound: 8 score matrices of [2048, 2048] need exp
  (262144 activation rows).  A single engine can't do that in under
  ~220us, so exp is split across ScalarE (native Exp -> fp8 P), DVE and
  GpSimd (1-instruction Schraudolph exp: int16 <- A*s + B, bitcast bf16).
  The per-(unit,comp) engine assignment table EXP_ASSIGN balances the
  three engines against their other duties.

  PE work is compressed with fp8 DoubleRow matmuls (0.5 cyc/row, two
  k-tiles per instruction): scores contract K=32 as [32,2] pairs with a
  zero-padded second pair on the kT side; PV contracts adjacent key-tile
  pairs of fp8 P against fp8 v; the v projection and output projection
  pair adjacent 128-row chunks of the contraction.  P from the
  Schraudolph engines is bf16, so those PV chains run as plain bf16
  matmuls (assignment table keeps the fp8 share high enough for PE).

  Normalization exploits RMSNorm scale-invariance: instead of dividing
  each component by its softmax denominator, multiply by the *other*
  component's denominator (u = r2*O1 - lam*r1*O2, same direction after
  RMS).  Rowsums come from tiny N=1 matmuls against a ones vector
  (essentially free on PE).  The per-(q,head) scalars are applied with
  broadcast (stride-0) tensor_tensor ops batched over all 4 heads, and
  lam rides along in a fused scalar_tensor_tensor.  The overall sign
  flip is folded into a negated Wo on the host; RMS eps is absorbed
  (u's scale makes it negligible); subln_w, (1-lambda_init), the fp8
  range scales (8x q/k, 16x v, 64x Wo) and 1/16 fp8-P offset all fold
  into host-side weight prep or activation scale/bias.
"""

import math
import sys

sys.path.insert(0, "/opt/trn_rl_repo")

from collections import deque
from contextlib import ExitStack

import ml_dtypes
import numpy as np

import concourse.bacc as bacc
import concourse.mybir as mybir
import concourse.tile as tile
from concourse.bass import AP, broadcast_tensor_aps
from concourse.bass_utils import run_bass_kernel_spmd

# The kernel's only transcendentals are Exp and Ln; make the activation
# table-set chooser prefer the one set containing both, so a single
# ACT_TABLE_LOAD covers the whole kernel.
_orig_get_activation_tables = bacc.get_activation_tables


def _tables_ln_exp_pinned(arch):
    t = dict(_orig_get_activation_tables(arch))
    pref = "natural_log_exp_and_others"
    if pref not in t:
        return t
    A = mybir.ActivationFunctionType
    out = {}
    for k, v in t.items():
        if k != pref:
            v = {f for f in v if f not in (A.Exp, A.Ln)}
        out[k] = v
    return out


bacc.get_activation_tables = _tables_ln_exp_pinned

F32 = mybir.dt.float32
BF16 = mybir.dt.bfloat16
FP8 = mybir.dt.float8e4
I16 = mybir.dt.int16
ALU = mybir.AluOpType
ACT = mybir.ActivationFunctionType
PM = mybir.MatmulPerfMode

E = 1024          # embed dim
S = 2048          # sequence length
B = 2             # batch
H = 16            # real heads
D = 32            # head dim (per component)
NCORES = 8
HPC = 4           # real heads per core
FPC = HPC * 2 * D  # features per core for q/k/v slices = 256
LAMBDA_INIT = 0.8 - 0.6 * math.exp(-0.3 * 12)
EPS = 1e-5

QC = 256          # query-chunk width
NQC = S // QC     # 8
NKT = S // 128    # 16 key tiles
NG = 4            # score groups per unit (4 ktiles each)

# fp8 range scales (folded into host weight prep)
QKSCALE = 8.0     # q and k each scaled 8x -> fp8-friendly
VSCALE = 16.0     # v scaled 16x (cancelled by RMS)
WOSCALE = 64.0    # Wo scaled 64x, removed in the psum->sbuf copy
ESC = (D ** -0.5) / (QKSCALE * QKSCALE)   # exp scale on raw score psum
LOG2E = 1.0 / math.log(2.0)
A16 = 128.0 * LOG2E * ESC                 # Schraudolph slope (bf16 bits)
B16 = 16251.0                             # tuned offset (max rel err 3.3%)

# exp engine per (unit_index, comp): 'A' ScalarE (fp8 P, DoubleRow PV),
# 'V' DVE, 'P' GpSimd (both bf16 P, plain PV).  64 entries, tuned so all
# three engines finish together.
def _make_assign(na, nv, npp):
    # interleave A/V/P counts evenly over 64 slots (largest remainder)
    total = na + nv + npp
    assert total == 64
    out = []
    cnt = {"A": 0, "V": 0, "P": 0}
    want = {"A": na, "V": nv, "P": npp}
    for i in range(64):
        # pick the engine furthest behind its quota
        best = max("AVP", key=lambda e: want[e] * (i + 1) / 64 - cnt[e])
        out.append(best)
        cnt[best] += 1
    return out


EXP_ASSIGN = _make_assign(40, 24, 0)


def build_kernel(reps: int = 1):
    nc = bacc.Bacc("TRN2", target_bir_lowering=False, debug=False,
                   num_devices=NCORES)
    xbf = nc.dram_tensor("xbf", [E, S], BF16, kind="ExternalInput")
    wq = nc.dram_tensor("wq", [E, FPC], BF16, kind="ExternalInput")
    wk = nc.dram_tensor("wk", [E, FPC], BF16, kind="ExternalInput")
    wv = nc.dram_tensor("wv", [E, FPC], BF16, kind="ExternalInput")
    wob = nc.dram_tensor("wob", [128, 2 * E], BF16, kind="ExternalInput")
    cf32 = nc.dram_tensor("cf32", [128, 1], F32, kind="ExternalInput")
    idb = nc.dram_tensor("idb", [128, 128], BF16, kind="ExternalInput")
    out = nc.dram_tensor("out", [S, E], F32, kind="ExternalOutput")
    DEBUG = getattr(build_kernel, "debug", False)
    if DEBUG:
        dbg_qt = nc.dram_tensor("dbg_qt", [128, S], FP8, kind="ExternalOutput")
        dbg_kt = nc.dram_tensor("dbg_kt", [128, 2 * S], FP8, kind="ExternalOutput")
        dbg_vt = nc.dram_tensor("dbg_vt", [128, NKT * FPC], FP8, kind="ExternalOutput")
        dbg_pt0 = nc.dram_tensor("dbg_pt0", [128, 4096], mybir.dt.uint8, kind="ExternalOutput")
        dbg_pt1 = nc.dram_tensor("dbg_pt1", [128, 4096], mybir.dt.uint8, kind="ExternalOutput")
        dbg_o = nc.dram_tensor("dbg_o", [128, 512], F32, kind="ExternalOutput")
        dbg_r = nc.dram_tensor("dbg_r", [128, 16], F32, kind="ExternalOutput")
        dbg_ab = nc.dram_tensor("dbg_ab", [128, 256], F32, kind="ExternalOutput")

    with tile.TileContext(nc) as tc, ExitStack() as ctx:
        cpool = ctx.enter_context(tc.tile_pool(name="consts", bufs=1))
        ipool = ctx.enter_context(tc.tile_pool(name="inputs", bufs=1))
        qpool = ctx.enter_context(tc.tile_pool(name="qkv", bufs=1))
        pt16p = ctx.enter_context(tc.tile_pool(name="pt16", bufs=2))
        wpool = ctx.enter_context(tc.tile_pool(name="work", bufs=3))
        pst = ctx.enter_context(tc.tile_pool(name="pst", bufs=2, space="PSUM"))
        po = ctx.enter_context(tc.tile_pool(name="po", bufs=2, space="PSUM"))
        pops = ctx.enter_context(tc.tile_pool(name="pops", bufs=1, space="PSUM"))
        pr = ctx.enter_context(tc.tile_pool(name="pr", bufs=1, space="PSUM"))

        # ---------------- consts ----------------
        lam_sb = cpool.tile([128, 1], F32, tag="lam")
        nc.sync.dma_start(lam_sb[:], cf32.ap())
        idb_sb = cpool.tile([128, 128], BF16, tag="idb")
        nc.sync.dma_start(idb_sb[:], idb.ap())
        eps_sb = cpool.tile([128, 1], F32, tag="eps")
        nc.vector.memset(eps_sb[:], EPS)
        ones_bf = cpool.tile([128, 1], BF16, tag="onesb")
        nc.vector.memset(ones_bf[:], 1.0)

        # ---------------- inputs ----------------
        xbf_sb = ipool.tile([128, 8, S], BF16, tag="xbf")
        wq_sb = ipool.tile([128, 8, FPC], BF16, tag="wq")
        wk_sb = ipool.tile([128, 8, FPC], BF16, tag="wk")
        wv_sb = ipool.tile([128, 8, FPC], BF16, tag="wv")
        wob_sb = ipool.tile([128, 2, E], BF16, tag="wob")
        wkr = wk.ap().rearrange("(kb p) f -> p kb f", p=128)
        nc.sync.dma_start(wk_sb[:, 0:4, :], wkr[:, 0:4, :])
        nc.sync.dma_start(wk_sb[:, 4:8, :], wkr[:, 4:8, :])
        for nch in range(4):
            for kb in range(8):
                eng = (nc.sync, nc.gpsimd)[kb % 2]
                eng.dma_start(
                    xbf_sb[:, kb, nch * 512:(nch + 1) * 512],
                    xbf.ap()[kb * 128:(kb + 1) * 128,
                             nch * 512:(nch + 1) * 512])
            if nch == 0:
                nc.sync.dma_start(
                    wq_sb[:], wq.ap().rearrange("(kb p) f -> p kb f", p=128))
            if nch == 1:
                nc.sync.dma_start(
                    wv_sb[:], wv.ap().rearrange("(kb p) f -> p kb f", p=128))
        nc.sync.dma_start(wob_sb[:], wob.ap())

        # ---------------- persistent qkv tiles ----------------
        # qT/kT: [feat, seq] fp8.  kT has a zero right half: the DoubleRow
        # score matmul pairs [32,2] along free; pair 1 reads zero weights.
        qT = [qpool.tile([128, S], BF16, tag=f"qT{fb}", name="qT")
              for fb in range(2)]
        kT = [qpool.tile([128, S], BF16, tag=f"kT{fb}", name="kT")
              for fb in range(2)]
        vt = qpool.tile([128, NKT * FPC], BF16, tag="vt", name="vt")

        for _rep in range(reps):
            # ------------- projection helpers -------------
            def proj_qk_round(dst, w_sb, fb, nch, copy_eng):
                ps = pops.tile([128, 512], F32, tag="ops", name="ops")
                for kb in range(8):
                    nc.tensor.matmul(
                        ps[:], w_sb[:, kb, fb * 128:(fb + 1) * 128],
                        xbf_sb[:, kb, nch * 512:(nch + 1) * 512],
                        start=(kb == 0), stop=(kb == 7))
                copy_eng.tensor_copy(dst[fb][:, nch * 512:(nch + 1) * 512], ps[:])

            def proj_v_round(st, copy_eng):
                ps = po.tile([128, 512], F32, tag="o", name="vps")
                for kb in range(8):
                    nc.tensor.matmul(
                        ps[:, 0:FPC],
                        xbf_sb[:, kb, st * 128:(st + 1) * 128],
                        wv_sb[:, kb, :],
                        start=(kb == 0), stop=(kb == 7))
                copy_eng.tensor_copy(vt[:, st * FPC:(st + 1) * FPC], ps[:, 0:FPC])

            # ------------- deferred-work scheduler -------------
            # slots at (unit, group) granularity; at(k, fn) runs fn k slots
            # from now.
            sched = deque([[] for _ in range(24)])

            def at(k, fn):
                sched[k].append(fn)

            def pop_slot():
                for fn in sched.popleft():
                    fn()
                sched.append([])

            # prologue: k/q fb0 nch0 only; later chunks land just in
            # time for the score groups that need them.
            proj_qk_round(kT, wk_sb, 0, 0, nc.vector)
            proj_qk_round(qT, wq_sb, 0, 0, nc.vector)

            # deferred projections: (fn, slot) list consumed by early units
            def mk_qk(dst, w_sb, fb, nch, eng):
                return lambda: proj_qk_round(dst, w_sb, fb, nch, eng)

            def mk_v(st, eng):
                return lambda: proj_v_round(st, eng)

            # v rounds during unit 0 (4 per group-slot, ahead of PV use);
            # k fb1 during units 0-1 (needed by unit 2); q rounds spread.
            dwork = {}  # slot index (absolute) -> list of fns

            def dq(slot, fn):
                dwork.setdefault(slot, []).append(fn)

            for st in range(8):
                dq(0, mk_v(st, nc.vector))
            for st in range(8, 16):
                dq(1, mk_v(st, nc.vector))
            dq(0, mk_qk(kT, wk_sb, 0, 1, nc.vector))
            dq(1, mk_qk(kT, wk_sb, 0, 2, nc.vector))
            dq(2, mk_qk(kT, wk_sb, 0, 3, nc.vector))
            for nch in range(4):
                dq(3 + nch, mk_qk(kT, wk_sb, 1, nch, nc.vector))
            dq(7, mk_qk(qT, wq_sb, 1, 0, nc.vector))
            # remaining q chunks: fb0 nch1-3 needed at qc2/4/6 (units 8/16/24
            # -> slots 32/64/96); fb1 similarly.
            dq(12, mk_qk(qT, wq_sb, 0, 1, nc.vector))
            dq(16, mk_qk(qT, wq_sb, 1, 1, nc.vector))
            dq(44, mk_qk(qT, wq_sb, 0, 2, nc.vector))
            dq(48, mk_qk(qT, wq_sb, 1, 2, nc.vector))
            dq(76, mk_qk(qT, wq_sb, 0, 3, nc.vector))
            dq(80, mk_qk(qT, wq_sb, 1, 3, nc.vector))

            # ------------- attention -------------
            units = [(qc, h) for qc in range(NQC) for h in range(HPC)]
            qc_state = {}
            slot_idx = 0

            def fill_half(u, g, c, hb):
                fb, off, qc = u["fb"], u["off"] + 32 * c, u["qc"]
                rhs = qT[fb][off:off + 32, qc * QC:(qc + 1) * QC]
                tp = (off, 0) if off == 96 else None
                st_ps = pst.tile([128, 512], F32, tag=f"st{hb}", name="st")
                for j in range(2):
                    ktile = 4 * g + 2 * hb + j
                    nc.tensor.matmul(
                        st_ps[:, j * QC:(j + 1) * QC],
                        kT[fb][off:off + 32,
                               ktile * 128:(ktile + 1) * 128],
                        rhs, start=True, stop=True, tile_position=tp)
                return st_ps

            def emit_exp(u, g, c, halves):
                eng = u["eng"][c]
                for hb in range(2):
                    sl = u["pt"][c][:, g * 1024 + hb * 512:
                                    g * 1024 + hb * 512 + 512]
                    if eng == "A":
                        nc.scalar.activation(sl, halves[hb][:], ACT.Exp,
                                             scale=ESC)
                    else:
                        nc.vector.tensor_scalar(
                            sl.bitcast(I16), halves[hb][:],
                            A16, B16, op0=ALU.mult, op1=ALU.add)

            def emit_pv(u, g, c):
                qc, h = u["qc"], u["h"]
                o_t = qc_state[qc]["O"]
                ptb = u["pt"][c][:]
                first_chain = (h == 0 and c == 0)
                for qs in range(2):
                    ot_ap = o_t[qs].rearrange("p (h c d) -> p h c d", c=2, d=64)
                    out_ap = ot_ap[:, h, c, :]
                    for jj in range(4):
                        j = 4 * g + jj
                        nc.tensor.matmul(
                            out_ap,
                            ptb[:, j * QC + qs * 128:j * QC + qs * 128 + 128],
                            vt[:, j * FPC + h * 64:j * FPC + h * 64 + 64],
                            start=(g == 0 and jj == 0 and first_chain),
                            stop=(g == NG - 1 and jj == 3),
                            skip_group_check=True)

            def emit_rowsum(u, g, c):
                qc, h = u["qc"], u["h"]
                r_t = qc_state[qc]["r"]
                ptb = u["pt"][c][:]
                for qs in range(2):
                    first_chain = (h == 0 and c == 0 and qs == 0)
                    col = qs * 8 + h * 2 + c
                    out_ap = r_t[:, col:col + 1]
                    for jj in range(4):
                        j = 4 * g + jj
                        nc.tensor.matmul(
                            out_ap,
                            ptb[:, j * QC + qs * 128:j * QC + qs * 128 + 128],
                            ones_bf[:, 0:1],
                            start=(g == 0 and jj == 0 and first_chain),
                            stop=(g == NG - 1 and jj == 3),
                            skip_group_check=True)

            # ------------- per-qc tail -------------
            def mk_normalize(qc, qs):
                def _fn():
                    stt = qc_state[qc]
                    if qs == 0:
                        rall = wpool.tile([128, 16], F32, tag="rall",
                                          name="rall")
                        stt["rall"] = rall
                        nc.vector.tensor_copy(rall[:], stt["r"][:])
                    rall = stt["rall"]
                    rv = rall.rearrange("p (s h c) -> p s h c", s=2, c=2)
                    o_t = stt["O"][qs]
                    ov = o_t.rearrange("p (h c d) -> p h c d", c=2, d=64)
                    t1 = wpool.tile([128, 4, 64], BF16, tag="t1", name="t1")
                    t2 = wpool.tile([128, 4, 64], BF16, tag="t2", name="t2")
                    uu = wpool.tile([128, 4, 64], BF16, tag=f"u{qs}", name="u")
                    s2 = wpool.tile([128, 4, 64], BF16, tag="s2", name="s2")
                    i0, i1 = broadcast_tensor_aps(ov[:, :, 0, :],
                                                  rv[:, qs, :, 1:2])
                    nc.vector.tensor_tensor(t1[:], i0, i1, op=ALU.mult)
                    i0, i1 = broadcast_tensor_aps(ov[:, :, 1, :],
                                                  rv[:, qs, :, 0:1])
                    nc.vector.tensor_tensor(t2[:], i0, i1, op=ALU.mult)
                    # u = lam*t2 - t1  (= -(r2 O1 - lam r1 O2); Wo negated)
                    nc.vector.scalar_tensor_tensor(
                        uu[:], t2[:], lam_sb[:, 0:1], t1[:],
                        op0=ALU.mult, op1=ALU.subtract)
                    nc.gpsimd.tensor_mul(s2[:], uu[:], uu[:])
                    nc.vector.tensor_reduce(
                        stt["ssq"][:, qs, :], s2[:],
                        axis=mybir.AxisListType.X, op=ALU.add)
                    stt[f"u{qs}"] = uu
                return _fn

            def mk_rms(qc):
                def _fn():
                    stt = qc_state[qc]
                    rln = wpool.tile([128, 8], F32, tag="rln", name="rln")
                    rmsi = wpool.tile([128, 8], BF16, tag="rmsi", name="rmsi")
                    nc.scalar.activation(rln[:], stt["ssq"][:].rearrange(
                        "p a b -> p (a b)"), ACT.Ln,
                        scale=1.0 / 64.0, bias=eps_sb[:, 0:1])
                    nc.scalar.activation(rmsi[:], rln[:], ACT.Exp, scale=-0.5)
                    stt["rmsi"] = rmsi
                return _fn

            def mk_apply_tp(qc, qs, pool=None):
                def _fn():
                    stt = qc_state[qc]
                    uu = stt[f"u{qs}"]
                    rmsi = stt["rmsi"].rearrange("p (s h) -> p s h", s=2)
                    ab = wpool.tile([128, 4, 64], BF16, tag=f"ab{qs}",
                                    name="ab")
                    i0, i1 = broadcast_tensor_aps(
                        uu[:], rmsi[:, qs, :].rearrange("p (h o) -> p h o",
                                                        o=1))
                    nc.gpsimd.tensor_tensor(ab[:], i0, i1, op=ALU.mult)
                    abf = ab.rearrange("p h d -> p (h d)")
                    atp = (pool or pops).tile([128, 512], F32,
                                              tag="ops" if pool is None
                                              else "o", name="atps")
                    atps = atp[:].bitcast(BF16)
                    for fc in range(2):
                        nc.tensor.transpose(
                            atps[:, fc * 136:fc * 136 + 128],
                            abf[:, fc * 128:(fc + 1) * 128], idb_sb[:])
                    atb = wpool.tile([128, 2, 128], BF16, tag=f"at{qs}",
                                     name="atb")
                    tsrc = atps[:, 0:272].rearrange(
                        "p (t f) -> p t f", t=2)[:, :, 0:128]
                    nc.vector.tensor_copy(atb[:], tsrc)
                    stt[f"at{qs}"] = atb
                return _fn

            def mk_oproj(qc, qs, ec, osb_eng, pool=None):
                def _fn():
                    stt = qc_state[qc]
                    atb = stt[f"at{qs}"]
                    ps = (pool or pops).tile([128, 512], F32,
                                             tag="ops" if pool is None
                                             else "o", name="ops")
                    for fc in range(2):
                        nc.tensor.matmul(
                            ps[:], atb[:, fc, :],
                            wob_sb[:, fc, ec * 512:(ec + 1) * 512],
                            start=(fc == 0), stop=(fc == 1))
                    osb = wpool.tile([128, 512], F32, tag="osb", name="osb")
                    if osb_eng is nc.scalar:
                        nc.scalar.copy(osb[:], ps[:])
                    else:
                        osb_eng.tensor_copy(osb[:], ps[:])
                    row = (qc * 2 + qs) * 128
                    nc.sync.dma_start(
                        out.ap()[row:row + 128, ec * 512:(ec + 1) * 512],
                        osb[:])
                return _fn

            def mk_dbg(qc):
                def _fn():
                    stt = qc_state[qc]
                    ou = wpool.tile([128, 512], F32, tag="dbgo", name="dbgo")
                    nc.vector.tensor_copy(ou[:], stt["O"][0][:])
                    nc.sync.dma_start(dbg_o.ap(), ou[:])
                    nc.sync.dma_start(dbg_r.ap(), stt["rall"][:])
                    ab = wpool.tile([128, 256], F32, tag="dbgab", name="dbgab")
                    nc.vector.tensor_copy(
                        ab.rearrange("p (h d) -> p h d", d=64)[:],
                        stt["u0"][:])
                    nc.sync.dma_start(dbg_ab.ap(), ab[:])
                return _fn

            for pi in range(len(units) // 2):
                pair = [units[2 * pi], units[2 * pi + 1]]
                qc = pair[0][0]
                if qc not in qc_state:
                    qc_state[qc] = {
                        "O": [po.tile([128, 512], F32, tag="o",
                                      name=f"O{qs}") for qs in range(2)],
                        "r": None,
                        "ssq": wpool.tile([128, 2, 4], F32, tag="ssq",
                                          name="ssq"),
                    }
                uu = []
                for k, (qc_, h_) in enumerate(pair):
                    ui = 2 * pi + k
                    uu.append({
                        "qc": qc_, "h": h_, "fb": h_ // 2,
                        "off": (h_ % 2) * 64,
                        "eng": (EXP_ASSIGN[2 * ui], EXP_ASSIGN[2 * ui + 1]),
                        "pt": [pt16p.tile([128, 4096], BF16,
                                          tag=f"pt{k}{c}", name="pt16")
                               for c in range(2)],
                    })
                st_cur = {}
                for hb in range(2):
                    for k in range(2):
                        for c in range(2):
                            st_cur.setdefault((k, c), []).append(
                                fill_half(uu[k], 0, c, hb))
                for g in range(NG):
                    if g == 0:
                        pop_slot()  # normalize(prev qc) ahead of exps on DVE
                    for k in range(2):
                        for c in range(2):
                            emit_exp(uu[k], g, c, st_cur[(k, c)])
                    if qc_state[qc]["r"] is None and g >= 1:
                        qc_state[qc]["r"] = pr.tile([128, 16], F32, tag="r",
                                                    name="r")
                    if g >= 1:
                        for k in range(2):
                            for c in range(2):
                                emit_rowsum(uu[k], g - 1, c)
                                emit_pv(uu[k], g - 1, c)
                    for fn in dwork.pop(slot_idx + 2 * g, []):
                        fn()
                    if g > 0:
                        pop_slot()
                    for fn in dwork.pop(slot_idx + 2 * g + 1, []):
                        fn()
                    pop_slot()
                    if g + 1 < NG:
                        st_cur = {}
                        for hb in range(2):
                            for k in range(2):
                                for c in range(2):
                                    st_cur.setdefault((k, c), []).append(
                                        fill_half(uu[k], g + 1, c, hb))
                for k in range(2):
                    for c in range(2):
                        emit_rowsum(uu[k], NG - 1, c)
                        emit_pv(uu[k], NG - 1, c)
                if DEBUG and pi == 7:
                    nc.sync.dma_start(dbg_qt.ap(), qT[0][:])
                    nc.sync.dma_start(dbg_kt.ap(), kT[0][:])
                    nc.sync.dma_start(dbg_vt.ap(), vt[:])
                if DEBUG and qc == 2 and pair[0][1] == 0:
                    for cc, dt_ in ((0, dbg_pt0), (1, dbg_pt1)):
                        tt = uu[0]["pt"][cc]
                        nc.sync.dma_start(
                            dt_.ap()[:, 0:4096],
                            tt[:].bitcast(mybir.dt.uint8)[:, 0:4096])
                if pair[1][1] == HPC - 1:
                    if DEBUG and qc == 2:
                        at(2, mk_dbg(qc))
                    last = qc == NQC - 1
                    pl = po if last else None
                    at(0, mk_normalize(qc, 0))
                    at(0 if last else 1, mk_normalize(qc, 1))
                    at(0 if last else 1, mk_rms(qc))
                    at(1 if last else 2, mk_apply_tp(qc, 0, pl))
                    at(1 if last else 2, mk_oproj(qc, 0, 0, nc.vector, pl))
                    at(2 if last else 3, mk_oproj(qc, 0, 1, nc.vector, pl))
                    at(2 if last else 3, mk_apply_tp(qc, 1, pl))
                    at(3 if last else 4, mk_oproj(qc, 1, 0, nc.vector, pl))
                    at(3 if last else 5, mk_oproj(qc, 1, 1, nc.vector, pl))
                slot_idx += 2 * NG
            # drain remaining scheduled work
            for fns in dwork.values():
                for fn in fns:
                    fn()
            while any(sched):
                pop_slot()
            qc_state.clear()
    nc.compile()
    return nc


def _prep_core_inputs(inputs, core):
    x = np.asarray(inputs["x"], np.float32)
    Wq = np.asarray(inputs["Wq"], np.float32)
    Wk = np.asarray(inputs["Wk"], np.float32)
    Wv = np.asarray(inputs["Wv"], np.float32)
    Wo = np.asarray(inputs["Wo"], np.float32)
    subln_w = np.asarray(inputs["subln_w"], np.float32)
    b, hg = core // 4, core % 4
    sl = slice(FPC * hg, FPC * (hg + 1))
    bf = ml_dtypes.bfloat16
    f8 = ml_dtypes.float8_e4m3
    lam_full = float(
        np.exp(np.sum(np.asarray(inputs["lambda_q1"], np.float64)
                      * np.asarray(inputs["lambda_k1"], np.float64)))
        - np.exp(np.sum(np.asarray(inputs["lambda_q2"], np.float64)
                        * np.asarray(inputs["lambda_k2"], np.float64)))
        + LAMBDA_INIT)
    xT = np.ascontiguousarray(x[b].T)
    wo_scale = (np.tile(subln_w, HPC) * (1.0 - LAMBDA_INIT))
    wo_dev = -(Wo[:, sl].T * wo_scale[:, None])
    wo_dev = np.ascontiguousarray(
        wo_dev.reshape(2, 128, E).transpose(1, 0, 2).reshape(128, 2 * E))
    return {
        "xbf": xT.astype(bf),
        "wq": np.ascontiguousarray(Wq[sl].T * QKSCALE).astype(bf),
        "wk": np.ascontiguousarray(Wk[sl].T * QKSCALE).astype(bf),
        "wv": np.ascontiguousarray(Wv[sl].T).astype(bf),
        "wob": wo_dev.astype(bf),
        "cf32": np.full((128, 1), lam_full, np.float32),
        "idb": np.eye(128, dtype=bf),
    }


_CACHED = {}


def _get_kernel(reps=1):
    if reps not in _CACHED:
        _CACHED[reps] = build_kernel(reps)
    return _CACHED[reps]


def run_on_cores(inputs, reps=1):
    nc = _get_kernel(reps)
    in_maps = [_prep_core_inputs(inputs, c) for c in range(NCORES)]
    res = run_bass_kernel_spmd(nc, in_maps, core_ids=list(range(NCORES)))
    return res


def kernel(**inputs) -> np.ndarray:
    res = run_on_cores(inputs)
    out = np.zeros((B, S, E), np.float32)
    for c in range(NCORES):
        out[c // 4] += res.results[c]["out"]
    return out


# revision 26
# speedup vs baseline: 1.0026x; 1.0026x over previous
"""Differential multi-head attention on 8 Trainium2 NeuronCores.

Sharding: tensor-parallel over heads x data-parallel over batch.
Core c handles batch b = c//4 and real heads [4*(c%4), 4*(c%4)+4).
Each core computes a partial output (its 256 attention features through
the output projection); the host sums the 4 partials per batch.

Per-core design (v2) -- three-engine exp + fp8 DoubleRow matmuls:

  The kernel is exp-bound: 8 score matrices of [2048, 2048] need exp
  (262144 activation rows).  A single engine can't do that in under
  ~220us, so exp is split across ScalarE (native Exp -> fp8 P), DVE and
  GpSimd (1-instruction Schraudolph exp: int16 <- A*s + B, bitcast bf16).
  The per-(unit,comp) engine assignment table EXP_ASSIGN balances the
  three engines against their other duties.

  PE work is compressed with fp8 DoubleRow matmuls (0.5 cyc/row, two
  k-tiles per instruction): scores contract K=32 as [32,2] pairs with a
  zero-padded second pair on the kT side; PV contracts adjacent key-tile
  pairs of fp8 P against fp8 v; the v projection and output projection
  pair adjacent 128-row chunks of the contraction.  P from the
  Schraudolph engines is bf16, so those PV chains run as plain bf16
  matmuls (assignment table keeps the fp8 share high enough for PE).

  Normalization exploits RMSNorm scale-invariance: instead of dividing
  each component by its softmax denominator, multiply by the *other*
  component's denominator (u = r2*O1 - lam*r1*O2, same direction after
  RMS).  Rowsums come from tiny N=1 matmuls against a ones vector
  (essentially free on PE).  The per-(q,head) scalars are applied with
  broadcast (stride-0) tensor_tensor ops batched over all 4 heads, and
  lam rides along in a fused scalar_tensor_tensor.  The overall sign
  flip is folded into a negated Wo on the host; RMS eps is absorbed
  (u's scale makes it negligible); subln_w, (1-lambda_init), the fp8
  range scales (8x q/k, 16x v, 64x Wo) and 1/16 fp8-P offset all fold
  into host-side weight prep or activation scale/bias.
"""

import math
import sys

sys.path.insert(0, "/opt/trn_rl_repo")

from collections import deque
from contextlib import ExitStack

import ml_dtypes
import numpy as np

import concourse.bacc as bacc
import concourse.mybir as mybir
import concourse.tile as tile
from concourse.bass import AP, broadcast_tensor_aps
from concourse.bass_utils import run_bass_kernel_spmd

# The kernel's only transcendentals are Exp and Ln; make the activation
# table-set chooser prefer the one set containing both, so a single
# ACT_TABLE_LOAD covers the whole kernel.
_orig_get_activation_tables = bacc.get_activation_tables


def _tables_ln_exp_pinned(arch):
    t = dict(_orig_get_activation_tables(arch))
    pref = "natural_log_exp_and_others"
    if pref not in t:
        return t
    A = mybir.ActivationFunctionType
    out = {}
    for k, v in t.items():
        if k != pref:
            v = {f for f in v if f not in (A.Exp, A.Ln)}
        out[k] = v
    return out


bacc.get_activation_tables = _tables_ln_exp_pinned

F32 = mybir.dt.float32
BF16 = mybir.dt.bfloat16
FP8 = mybir.dt.float8e4
I16 = mybir.dt.int16
ALU = mybir.AluOpType
ACT = mybir.ActivationFunctionType
PM = mybir.MatmulPerfMode

E = 1024          # embed dim
S = 2048          # sequence length
B = 2             # batch
H = 16            # real heads
D = 32            # head dim (per component)
NCORES = 8
HPC = 4           # real heads per core
FPC = HPC * 2 * D  # features per core for q/k/v slices = 256
LAMBDA_INIT = 0.8 - 0.6 * math.exp(-0.3 * 12)
EPS = 1e-5

QC = 256          # query-chunk width
NQC = S // QC     # 8
NKT = S // 128    # 16 key tiles
NG = 4            # score groups per unit (4 ktiles each)

# fp8 range scales (folded into host weight prep)
QKSCALE = 8.0     # q and k each scaled 8x -> fp8-friendly
VSCALE = 16.0     # v scaled 16x (cancelled by RMS)
WOSCALE = 64.0    # Wo scaled 64x, removed in the psum->sbuf copy
ESC = (D ** -0.5) / (QKSCALE * QKSCALE)   # exp scale on raw score psum
LOG2E = 1.0 / math.log(2.0)
A16 = 128.0 * LOG2E * ESC                 # Schraudolph slope (bf16 bits)
B16 = 16251.0                             # tuned offset (max rel err 3.3%)

# exp engine per (unit_index, comp): 'A' ScalarE (fp8 P, DoubleRow PV),
# 'V' DVE, 'P' GpSimd (both bf16 P, plain PV).  64 entries, tuned so all
# three engines finish together.
def _make_assign(na, nv, npp):
    # interleave A/V/P counts evenly over 64 slots (largest remainder)
    total = na + nv + npp
    assert total == 64
    out = []
    cnt = {"A": 0, "V": 0, "P": 0}
    want = {"A": na, "V": nv, "P": npp}
    for i in range(64):
        # pick the engine furthest behind its quota
        best = max("AVP", key=lambda e: want[e] * (i + 1) / 64 - cnt[e])
        out.append(best)
        cnt[best] += 1
    return out


EXP_ASSIGN = _make_assign(40, 24, 0)


def build_kernel(reps: int = 1):
    nc = bacc.Bacc("TRN2", target_bir_lowering=False, debug=False,
                   num_devices=NCORES)
    xbf = nc.dram_tensor("xbf", [E, S], BF16, kind="ExternalInput")
    wq = nc.dram_tensor("wq", [E, FPC], BF16, kind="ExternalInput")
    wk = nc.dram_tensor("wk", [E, FPC], BF16, kind="ExternalInput")
    wv = nc.dram_tensor("wv", [E, FPC], BF16, kind="ExternalInput")
    wob = nc.dram_tensor("wob", [128, 2 * E], BF16, kind="ExternalInput")
    cf32 = nc.dram_tensor("cf32", [128, 1], F32, kind="ExternalInput")
    idb = nc.dram_tensor("idb", [128, 128], BF16, kind="ExternalInput")
    out = nc.dram_tensor("out", [S, E], F32, kind="ExternalOutput")
    DEBUG = getattr(build_kernel, "debug", False)
    if DEBUG:
        dbg_qt = nc.dram_tensor("dbg_qt", [128, S], FP8, kind="ExternalOutput")
        dbg_kt = nc.dram_tensor("dbg_kt", [128, 2 * S], FP8, kind="ExternalOutput")
        dbg_vt = nc.dram_tensor("dbg_vt", [128, NKT * FPC], FP8, kind="ExternalOutput")
        dbg_pt0 = nc.dram_tensor("dbg_pt0", [128, 4096], mybir.dt.uint8, kind="ExternalOutput")
        dbg_pt1 = nc.dram_tensor("dbg_pt1", [128, 4096], mybir.dt.uint8, kind="ExternalOutput")
        dbg_o = nc.dram_tensor("dbg_o", [128, 512], F32, kind="ExternalOutput")
        dbg_r = nc.dram_tensor("dbg_r", [128, 16], F32, kind="ExternalOutput")
        dbg_ab = nc.dram_tensor("dbg_ab", [128, 256], F32, kind="ExternalOutput")

    with tile.TileContext(nc) as tc, ExitStack() as ctx:
        cpool = ctx.enter_context(tc.tile_pool(name="consts", bufs=1))
        ipool = ctx.enter_context(tc.tile_pool(name="inputs", bufs=1))
        qpool = ctx.enter_context(tc.tile_pool(name="qkv", bufs=1))
        pt16p = ctx.enter_context(tc.tile_pool(name="pt16", bufs=2))
        wpool = ctx.enter_context(tc.tile_pool(name="work", bufs=3))
        pst = ctx.enter_context(tc.tile_pool(name="pst", bufs=2, space="PSUM"))
        po = ctx.enter_context(tc.tile_pool(name="po", bufs=2, space="PSUM"))
        pops = ctx.enter_context(tc.tile_pool(name="pops", bufs=1, space="PSUM"))
        pr = ctx.enter_context(tc.tile_pool(name="pr", bufs=1, space="PSUM"))

        # ---------------- consts ----------------
        lam_sb = cpool.tile([128, 1], F32, tag="lam")
        nc.sync.dma_start(lam_sb[:], cf32.ap())
        idb_sb = cpool.tile([128, 128], BF16, tag="idb")
        nc.sync.dma_start(idb_sb[:], idb.ap())
        eps_sb = cpool.tile([128, 1], F32, tag="eps")
        nc.vector.memset(eps_sb[:], EPS)
        ones_bf = cpool.tile([128, 1], BF16, tag="onesb")
        nc.vector.memset(ones_bf[:], 1.0)

        # ---------------- inputs ----------------
        xbf_sb = ipool.tile([128, 8, S], BF16, tag="xbf")
        wq_sb = ipool.tile([128, 8, FPC], BF16, tag="wq")
        wk_sb = ipool.tile([128, 8, FPC], BF16, tag="wk")
        wv_sb = ipool.tile([128, 8, FPC], BF16, tag="wv")
        wob_sb = ipool.tile([128, 2, E], BF16, tag="wob")
        wkr = wk.ap().rearrange("(kb p) f -> p kb f", p=128)
        nc.sync.dma_start(wk_sb[:, 0:4, :], wkr[:, 0:4, :])
        nc.sync.dma_start(wk_sb[:, 4:8, :], wkr[:, 4:8, :])
        for nch in range(4):
            for kb in range(8):
                eng = (nc.sync, nc.gpsimd)[kb % 2]
                eng.dma_start(
                    xbf_sb[:, kb, nch * 512:(nch + 1) * 512],
                    xbf.ap()[kb * 128:(kb + 1) * 128,
                             nch * 512:(nch + 1) * 512])
            if nch == 0:
                nc.sync.dma_start(
                    wq_sb[:], wq.ap().rearrange("(kb p) f -> p kb f", p=128))
            if nch == 1:
                nc.sync.dma_start(
                    wv_sb[:], wv.ap().rearrange("(kb p) f -> p kb f", p=128))
        nc.sync.dma_start(wob_sb[:], wob.ap())

        # ---------------- persistent qkv tiles ----------------
        # qT/kT: [feat, seq] fp8.  kT has a zero right half: the DoubleRow
        # score matmul pairs [32,2] along free; pair 1 reads zero weights.
        qT = [qpool.tile([128, S], BF16, tag=f"qT{fb}", name="qT")
              for fb in range(2)]
        kT = [qpool.tile([128, S], BF16, tag=f"kT{fb}", name="kT")
              for fb in range(2)]
        vt = qpool.tile([128, NKT * FPC], BF16, tag="vt", name="vt")

        for _rep in range(reps):
            # ------------- projection helpers -------------
            def proj_qk_round(dst, w_sb, fb, nch, copy_eng):
                ps = pops.tile([128, 512], F32, tag="ops", name="ops")
                for kb in range(8):
                    nc.tensor.matmul(
                        ps[:], w_sb[:, kb, fb * 128:(fb + 1) * 128],
                        xbf_sb[:, kb, nch * 512:(nch + 1) * 512],
                        start=(kb == 0), stop=(kb == 7))
                copy_eng.tensor_copy(dst[fb][:, nch * 512:(nch + 1) * 512], ps[:])

            def proj_v_round(st, copy_eng):
                ps = po.tile([128, 512], F32, tag="o", name="vps")
                for kb in range(8):
                    nc.tensor.matmul(
                        ps[:, 0:FPC],
                        xbf_sb[:, kb, st * 128:(st + 1) * 128],
                        wv_sb[:, kb, :],
                        start=(kb == 0), stop=(kb == 7))
                copy_eng.tensor_copy(vt[:, st * FPC:(st + 1) * FPC], ps[:, 0:FPC])

            # ------------- deferred-work scheduler -------------
            # slots at (unit, group) granularity; at(k, fn) runs fn k slots
            # from now.
            sched = deque([[] for _ in range(24)])

            def at(k, fn):
                sched[k].append(fn)

            def pop_slot():
                for fn in sched.popleft():
                    fn()
                sched.append([])

            # prologue: k/q fb0 nch0 only; later chunks land just in
            # time for the score groups that need them.
            proj_qk_round(kT, wk_sb, 0, 0, nc.vector)
            proj_qk_round(qT, wq_sb, 0, 0, nc.vector)

            # deferred projections: (fn, slot) list consumed by early units
            def mk_qk(dst, w_sb, fb, nch, eng):
                return lambda: proj_qk_round(dst, w_sb, fb, nch, eng)

            def mk_v(st, eng):
                return lambda: proj_v_round(st, eng)

            # v rounds during unit 0 (4 per group-slot, ahead of PV use);
            # k fb1 during units 0-1 (needed by unit 2); q rounds spread.
            dwork = {}  # slot index (absolute) -> list of fns

            def dq(slot, fn):
                dwork.setdefault(slot, []).append(fn)

            for st in range(8):
                dq(0, mk_v(st, nc.vector))
            for st in range(8, 16):
                dq(1, mk_v(st, nc.vector))
            dq(0, mk_qk(kT, wk_sb, 0, 1, nc.vector))
            dq(1, mk_qk(kT, wk_sb, 0, 2, nc.vector))
            dq(2, mk_qk(kT, wk_sb, 0, 3, nc.vector))
            for nch in range(4):
                dq(3 + nch, mk_qk(kT, wk_sb, 1, nch, nc.vector))
            dq(7, mk_qk(qT, wq_sb, 1, 0, nc.vector))
            # remaining q chunks: fb0 nch1-3 needed at qc2/4/6 (units 8/16/24
            # -> slots 32/64/96); fb1 similarly.
            dq(12, mk_qk(qT, wq_sb, 0, 1, nc.vector))
            dq(16, mk_qk(qT, wq_sb, 1, 1, nc.vector))
            dq(44, mk_qk(qT, wq_sb, 0, 2, nc.vector))
            dq(48, mk_qk(qT, wq_sb, 1, 2, nc.vector))
            dq(76, mk_qk(qT, wq_sb, 0, 3, nc.vector))
            dq(80, mk_qk(qT, wq_sb, 1, 3, nc.vector))

            # ------------- attention -------------
            units = [(qc, h) for qc in range(NQC) for h in range(HPC)]
            qc_state = {}
            slot_idx = 0

            def fill_half(u, g, c, hb):
                fb, off, qc = u["fb"], u["off"] + 32 * c, u["qc"]
                rhs = qT[fb][off:off + 32, qc * QC:(qc + 1) * QC]
                tp = (off, 0) if off == 96 else None
                st_ps = pst.tile([128, 512], F32, tag=f"st{hb}", name="st")
                for j in range(2):
                    ktile = 4 * g + 2 * hb + j
                    nc.tensor.matmul(
                        st_ps[:, j * QC:(j + 1) * QC],
                        kT[fb][off:off + 32,
                               ktile * 128:(ktile + 1) * 128],
                        rhs, start=True, stop=True, tile_position=tp)
                return st_ps

            def emit_exp(u, g, c, halves):
                eng = u["eng"][c]
                for hb in range(2):
                    sl = u["pt"][c][:, g * 1024 + hb * 512:
                                    g * 1024 + hb * 512 + 512]
                    if eng == "A":
                        nc.scalar.activation(sl, halves[hb][:], ACT.Exp,
                                             scale=ESC)
                    else:
                        nc.vector.tensor_scalar(
                            sl.bitcast(I16), halves[hb][:],
                            A16, B16, op0=ALU.mult, op1=ALU.add)

            def emit_pv(u, g, c):
                qc, h = u["qc"], u["h"]
                o_t = qc_state[qc]["O"]
                ptb = u["pt"][c][:]
                first_chain = (h == 0 and c == 0)
                for qs in range(2):
                    ot_ap = o_t[qs].rearrange("p (h c d) -> p h c d", c=2, d=64)
                    out_ap = ot_ap[:, h, c, :]
                    for jj in range(4):
                        j = 4 * g + jj
                        nc.tensor.matmul(
                            out_ap,
                            ptb[:, j * QC + qs * 128:j * QC + qs * 128 + 128],
                            vt[:, j * FPC + h * 64:j * FPC + h * 64 + 64],
                            start=(g == 0 and jj == 0 and first_chain),
                            stop=(g == NG - 1 and jj == 3),
                            skip_group_check=True)

            def emit_rowsum(u, g, c):
                qc, h = u["qc"], u["h"]
                r_t = qc_state[qc]["r"]
                ptb = u["pt"][c][:]
                for qs in range(2):
                    first_chain = (h == 0 and c == 0 and qs == 0)
                    col = qs * 8 + h * 2 + c
                    out_ap = r_t[:, col:col + 1]
                    for jj in range(4):
                        j = 4 * g + jj
                        nc.tensor.matmul(
                            out_ap,
                            ptb[:, j * QC + qs * 128:j * QC + qs * 128 + 128],
                            ones_bf[:, 0:1],
                            start=(g == 0 and jj == 0 and first_chain),
                            stop=(g == NG - 1 and jj == 3),
                            skip_group_check=True)

            # ------------- per-qc tail -------------
            def mk_normalize(qc, qs):
                def _fn():
                    stt = qc_state[qc]
                    if qs == 0:
                        rall = wpool.tile([128, 16], F32, tag="rall",
                                          name="rall")
                        stt["rall"] = rall
                        nc.vector.tensor_copy(rall[:], stt["r"][:])
                    rall = stt["rall"]
                    rv = rall.rearrange("p (s h c) -> p s h c", s=2, c=2)
                    o_t = stt["O"][qs]
                    ov = o_t.rearrange("p (h c d) -> p h c d", c=2, d=64)
                    t1 = wpool.tile([128, 4, 64], BF16, tag="t1", name="t1")
                    t2 = wpool.tile([128, 4, 64], BF16, tag="t2", name="t2")
                    uu = wpool.tile([128, 4, 64], BF16, tag=f"u{qs}", name="u")
                    s2 = wpool.tile([128, 4, 64], BF16, tag="s2", name="s2")
                    i0, i1 = broadcast_tensor_aps(ov[:, :, 0, :],
                                                  rv[:, qs, :, 1:2])
                    nc.vector.tensor_tensor(t1[:], i0, i1, op=ALU.mult)
                    i0, i1 = broadcast_tensor_aps(ov[:, :, 1, :],
                                                  rv[:, qs, :, 0:1])
                    nc.vector.tensor_tensor(t2[:], i0, i1, op=ALU.mult)
                    # u = lam*t2 - t1  (= -(r2 O1 - lam r1 O2); Wo negated)
                    nc.vector.scalar_tensor_tensor(
                        uu[:], t2[:], lam_sb[:, 0:1], t1[:],
                        op0=ALU.mult, op1=ALU.subtract)
                    nc.gpsimd.tensor_mul(s2[:], uu[:], uu[:])
                    nc.vector.tensor_reduce(
                        stt["ssq"][:, qs, :], s2[:],
                        axis=mybir.AxisListType.X, op=ALU.add)
                    stt[f"u{qs}"] = uu
                return _fn

            def mk_rms(qc, qs=None):
                def _fn():
                    stt = qc_state[qc]
                    if qs is None:
                        rln = wpool.tile([128, 8], F32, tag="rln", name="rln")
                        rmsi = wpool.tile([128, 8], BF16, tag="rmsi",
                                          name="rmsi")
                        nc.scalar.activation(rln[:], stt["ssq"][:].rearrange(
                            "p a b -> p (a b)"), ACT.Ln,
                            scale=1.0 / 64.0, bias=eps_sb[:, 0:1])
                        nc.scalar.activation(rmsi[:], rln[:], ACT.Exp,
                                             scale=-0.5)
                        stt["rmsi"] = rmsi
                        return
                    if "rmsi" not in stt:
                        stt["rmsi"] = wpool.tile([128, 8], BF16, tag="rmsi",
                                                 name="rmsi")
                    rln = wpool.tile([128, 4], F32, tag="rln4", name="rln")
                    nc.scalar.activation(rln[:], stt["ssq"][:, qs, :],
                                         ACT.Ln, scale=1.0 / 64.0,
                                         bias=eps_sb[:, 0:1])
                    nc.scalar.activation(
                        stt["rmsi"][:, qs * 4:(qs + 1) * 4], rln[:],
                        ACT.Exp, scale=-0.5)
                return _fn

            def mk_apply_tp(qc, qs, pool=None, tt_eng=None):
                def _fn():
                    stt = qc_state[qc]
                    uu = stt[f"u{qs}"]
                    rmsi = stt["rmsi"].rearrange("p (s h) -> p s h", s=2)
                    ab = wpool.tile([128, 4, 64], BF16, tag=f"ab{qs}",
                                    name="ab")
                    i0, i1 = broadcast_tensor_aps(
                        uu[:], rmsi[:, qs, :].rearrange("p (h o) -> p h o",
                                                        o=1))
                    (tt_eng or nc.gpsimd).tensor_tensor(ab[:], i0, i1,
                                                       op=ALU.mult)
                    abf = ab.rearrange("p h d -> p (h d)")
                    atp = (pool or pops).tile([128, 512], F32,
                                              tag="ops" if pool is None
                                              else "o", name="atps")
                    atps = atp[:].bitcast(BF16)
                    for fc in range(2):
                        nc.tensor.transpose(
                            atps[:, fc * 136:fc * 136 + 128],
                            abf[:, fc * 128:(fc + 1) * 128], idb_sb[:])
                    atb = wpool.tile([128, 2, 128], BF16, tag=f"at{qs}",
                                     name="atb")
                    tsrc = atps[:, 0:272].rearrange(
                        "p (t f) -> p t f", t=2)[:, :, 0:128]
                    nc.vector.tensor_copy(atb[:], tsrc)
                    stt[f"at{qs}"] = atb
                return _fn

            def mk_oproj(qc, qs, ec, osb_eng, pool=None):
                def _fn():
                    stt = qc_state[qc]
                    atb = stt[f"at{qs}"]
                    ps = (pool or pops).tile([128, 512], F32,
                                             tag="ops" if pool is None
                                             else "o", name="ops")
                    for fc in range(2):
                        nc.tensor.matmul(
                            ps[:], atb[:, fc, :],
                            wob_sb[:, fc, ec * 512:(ec + 1) * 512],
                            start=(fc == 0), stop=(fc == 1))
                    osb = wpool.tile([128, 512], F32, tag="osb", name="osb")
                    if osb_eng is nc.scalar:
                        nc.scalar.copy(osb[:], ps[:])
                    else:
                        osb_eng.tensor_copy(osb[:], ps[:])
                    row = (qc * 2 + qs) * 128
                    nc.sync.dma_start(
                        out.ap()[row:row + 128, ec * 512:(ec + 1) * 512],
                        osb[:])
                return _fn

            def mk_dbg(qc):
                def _fn():
                    stt = qc_state[qc]
                    ou = wpool.tile([128, 512], F32, tag="dbgo", name="dbgo")
                    nc.vector.tensor_copy(ou[:], stt["O"][0][:])
                    nc.sync.dma_start(dbg_o.ap(), ou[:])
                    nc.sync.dma_start(dbg_r.ap(), stt["rall"][:])
                    ab = wpool.tile([128, 256], F32, tag="dbgab", name="dbgab")
                    nc.vector.tensor_copy(
                        ab.rearrange("p (h d) -> p h d", d=64)[:],
                        stt["u0"][:])
                    nc.sync.dma_start(dbg_ab.ap(), ab[:])
                return _fn

            for pi in range(len(units) // 2):
                pair = [units[2 * pi], units[2 * pi + 1]]
                qc = pair[0][0]
                if qc not in qc_state:
                    qc_state[qc] = {
                        "O": [po.tile([128, 512], F32, tag="o",
                                      name=f"O{qs}") for qs in range(2)],
                        "r": None,
                        "ssq": wpool.tile([128, 2, 4], F32, tag="ssq",
                                          name="ssq"),
                    }
                uu = []
                for k, (qc_, h_) in enumerate(pair):
                    ui = 2 * pi + k
                    uu.append({
                        "qc": qc_, "h": h_, "fb": h_ // 2,
                        "off": (h_ % 2) * 64,
                        "eng": (EXP_ASSIGN[2 * ui], EXP_ASSIGN[2 * ui + 1]),
                        "pt": [pt16p.tile([128, 4096], BF16,
                                          tag=f"pt{k}{c}", name="pt16")
                               for c in range(2)],
                    })
                st_cur = {}
                for hb in range(2):
                    for k in range(2):
                        for c in range(2):
                            st_cur.setdefault((k, c), []).append(
                                fill_half(uu[k], 0, c, hb))
                for g in range(NG):
                    if g == 0:
                        pop_slot()  # normalize(prev qc) ahead of exps on DVE
                    for k in range(2):
                        for c in range(2):
                            emit_exp(uu[k], g, c, st_cur[(k, c)])
                    if qc_state[qc]["r"] is None and g >= 1:
                        qc_state[qc]["r"] = pr.tile([128, 16], F32, tag="r",
                                                    name="r")
                    if g >= 1:
                        for k in range(2):
                            for c in range(2):
                                emit_rowsum(uu[k], g - 1, c)
                                emit_pv(uu[k], g - 1, c)
                    for fn in dwork.pop(slot_idx + 2 * g, []):
                        fn()
                    if g > 0:
                        pop_slot()
                    for fn in dwork.pop(slot_idx + 2 * g + 1, []):
                        fn()
                    pop_slot()
                    if g + 1 < NG:
                        st_cur = {}
                        for hb in range(2):
                            for k in range(2):
                                for c in range(2):
                                    st_cur.setdefault((k, c), []).append(
                                        fill_half(uu[k], g + 1, c, hb))
                for k in range(2):
                    for c in range(2):
                        emit_rowsum(uu[k], NG - 1, c)
                        emit_pv(uu[k], NG - 1, c)
                if DEBUG and pi == 7:
                    nc.sync.dma_start(dbg_qt.ap(), qT[0][:])
                    nc.sync.dma_start(dbg_kt.ap(), kT[0][:])
                    nc.sync.dma_start(dbg_vt.ap(), vt[:])
                if DEBUG and qc == 2 and pair[0][1] == 0:
                    for cc, dt_ in ((0, dbg_pt0), (1, dbg_pt1)):
                        tt = uu[0]["pt"][cc]
                        nc.sync.dma_start(
                            dt_.ap()[:, 0:4096],
                            tt[:].bitcast(mybir.dt.uint8)[:, 0:4096])
                if pair[1][1] == HPC - 1:
                    if DEBUG and qc == 2:
                        at(2, mk_dbg(qc))
                    if qc == NQC - 1:
                        at(0, mk_normalize(qc, 0))
                        at(0, mk_rms(qc, 0))
                        at(0, mk_apply_tp(qc, 0, po, nc.vector))
                        at(0, mk_oproj(qc, 0, 0, nc.vector, po))
                        at(0, mk_normalize(qc, 1))
                        at(0, mk_rms(qc, 1))
                        at(0, mk_oproj(qc, 0, 1, nc.scalar, po))
                        at(0, mk_apply_tp(qc, 1, po, nc.gpsimd))
                        at(1, mk_oproj(qc, 1, 0, nc.vector, po))
                        at(1, mk_oproj(qc, 1, 1, nc.scalar, po))
                    else:
                        at(0, mk_normalize(qc, 0))
                        at(1, mk_normalize(qc, 1))
                        at(1, mk_rms(qc))
                        at(2, mk_apply_tp(qc, 0))
                        at(2, mk_oproj(qc, 0, 0, nc.vector))
                        at(3, mk_oproj(qc, 0, 1, nc.vector))
                        at(3, mk_apply_tp(qc, 1))
                        at(4, mk_oproj(qc, 1, 0, nc.vector))
                        at(5, mk_oproj(qc, 1, 1, nc.vector))
                slot_idx += 2 * NG
            # drain remaining scheduled work
            for fns in dwork.values():
                for fn in fns:
                    fn()
            while any(sched):
                pop_slot()
            qc_state.clear()
    nc.compile()
    return nc


def _prep_core_inputs(inputs, core):
    x = np.asarray(inputs["x"], np.float32)
    Wq = np.asarray(inputs["Wq"], np.float32)
    Wk = np.asarray(inputs["Wk"], np.float32)
    Wv = np.asarray(inputs["Wv"], np.float32)
    Wo = np.asarray(inputs["Wo"], np.float32)
    subln_w = np.asarray(inputs["subln_w"], np.float32)
    b, hg = core // 4, core % 4
    sl = slice(FPC * hg, FPC * (hg + 1))
    bf = ml_dtypes.bfloat16
    f8 = ml_dtypes.float8_e4m3
    lam_full = float(
        np.exp(np.sum(np.asarray(inputs["lambda_q1"], np.float64)
                      * np.asarray(inputs["lambda_k1"], np.float64)))
        - np.exp(np.sum(np.asarray(inputs["lambda_q2"], np.float64)
                        * np.asarray(inputs["lambda_k2"], np.float64)))
        + LAMBDA_INIT)
    xT = np.ascontiguousarray(x[b].T)
    wo_scale = (np.tile(subln_w, HPC) * (1.0 - LAMBDA_INIT))
    wo_dev = -(Wo[:, sl].T * wo_scale[:, None])
    wo_dev = np.ascontiguousarray(
        wo_dev.reshape(2, 128, E).transpose(1, 0, 2).reshape(128, 2 * E))
    return {
        "xbf": xT.astype(bf),
        "wq": np.ascontiguousarray(Wq[sl].T * QKSCALE).astype(bf),
        "wk": np.ascontiguousarray(Wk[sl].T * QKSCALE).astype(bf),
        "wv": np.ascontiguousarray(Wv[sl].T).astype(bf),
        "wob": wo_dev.astype(bf),
        "cf32": np.full((128, 1), lam_full, np.float32),
        "idb": np.eye(128, dtype=bf),
    }


_CACHED = {}


def _get_kernel(reps=1):
    if reps not in _CACHED:
        _CACHED[reps] = build_kernel(reps)
    return _CACHED[reps]


def run_on_cores(inputs, reps=1):
    nc = _get_kernel(reps)
    in_maps = [_prep_core_inputs(inputs, c) for c in range(NCORES)]
    res = run_bass_kernel_spmd(nc, in_maps, core_ids=list(range(NCORES)))
    return res


def kernel(**inputs) -> np.ndarray:
    res = run_on_cores(inputs)
    out = np.zeros((B, S, E), np.float32)
    for c in range(NCORES):
        out[c // 4] += res.results[c]["out"]
    return out


# revision 30
# speedup vs baseline: 1.0080x; 1.0054x over previous
"""Differential multi-head attention on 8 Trainium2 NeuronCores.

Sharding: tensor-parallel over heads x data-parallel over batch.
Core c handles batch b = c//4 and real heads [4*(c%4), 4*(c%4)+4).
Each core computes a partial output (its 256 attention features through
the output projection); the host sums the 4 partials per batch.

Per-core design (v2): dual-engine exp + multiplicative RMS normalization.

  The kernel is jointly bound by PE matmuls (~224us bf16 floor) and exp
  over 8 score matrices of [2048, 2048] (262144 activation rows, ~265us
  if one engine did it all).  exp is split between ScalarE (native Exp,
  ~62% of rows) and DVE (1-instruction Schraudolph exp: int16 bits <-
  A*s + B, bitcast to bf16; max rel err 3.3%), per the tuned EXP_ASSIGN
  pattern.  All matmuls are bf16 (fp8/DoubleRow was tried everywhere
  and rejected: e4m3's 3-6% quantization on q/k/v/attn/Wo and its
  240-max range vs scores reaching +-10.4 blow the 2e-2 error budget).

  Normalization exploits RMSNorm scale-invariance to avoid divisions:
  u = r2*O1 - lam*r1*O2 points the same way as O1/r1 - lam*O2/r2 after
  RMS.  Rowsums r come from N=1 matmuls against a ones vector
  (essentially free on PE), lam rides in a fused scalar_tensor_tensor,
  and the per-(q,head) scalars apply via stride-0 broadcast
  tensor_tensor ops batched over all 4 heads.  The square step runs on
  GpSimd (SBUF-only; GPSIMD cannot touch PSUM).  The sign flip is
  folded into a negated Wo on the host; RMS eps is absorbed by u's
  scale; subln_w and (1-lambda_init) fold into Wo.

  Pipelining: units (qc, head) are processed in interleaved PAIRS so
  each exp engine's queue stays back-to-back while the other unit's
  score fills rotate through PSUM.  Score groups use two 1-bank
  [128, 512] half-tiles (tags st0/st1, bufs=2) so a half's fill->exp
  dependency releases independently; PV and rowsum chains lag one
  group as always-ready PE filler.  PSUM: 4 banks scores + 2 banks O
  accumulators (shared with out-proj psum via pool rotation) + 1 bank
  out-proj/transposes + 1 bank rowsums.  The x DMA lands in seq-major
  [128, 512] chunks so the first score group starts after ~1/4 of x;
  weights use single rearranged-AP DMAs.  The final chunk's
  normalize/out-proj chain is special-cased into the freed O banks
  with per-qsub rms to shorten the drain.

Modeled per-core time (TRN2 InstructionCostModel): ~266us
(PE busy ~225us, ScalarE ~200us, DVE ~193us).
"""

import math
import sys

sys.path.insert(0, "/opt/trn_rl_repo")

from collections import deque
from contextlib import ExitStack

import ml_dtypes
import numpy as np

import concourse.bacc as bacc
import concourse.mybir as mybir
import concourse.tile as tile
from concourse.bass import AP, broadcast_tensor_aps
from concourse.bass_utils import run_bass_kernel_spmd

# The kernel's only transcendentals are Exp and Ln; make the activation
# table-set chooser prefer the one set containing both, so a single
# ACT_TABLE_LOAD covers the whole kernel.
_orig_get_activation_tables = bacc.get_activation_tables


def _tables_ln_exp_pinned(arch):
    t = dict(_orig_get_activation_tables(arch))
    pref = "natural_log_exp_and_others"
    if pref not in t:
        return t
    A = mybir.ActivationFunctionType
    out = {}
    for k, v in t.items():
        if k != pref:
            v = {f for f in v if f not in (A.Exp, A.Ln)}
        out[k] = v
    return out


bacc.get_activation_tables = _tables_ln_exp_pinned

F32 = mybir.dt.float32
BF16 = mybir.dt.bfloat16
FP8 = mybir.dt.float8e4
I16 = mybir.dt.int16
ALU = mybir.AluOpType
ACT = mybir.ActivationFunctionType
PM = mybir.MatmulPerfMode

E = 1024          # embed dim
S = 2048          # sequence length
B = 2             # batch
H = 16            # real heads
D = 32            # head dim (per component)
NCORES = 8
HPC = 4           # real heads per core
FPC = HPC * 2 * D  # features per core for q/k/v slices = 256
LAMBDA_INIT = 0.8 - 0.6 * math.exp(-0.3 * 12)
EPS = 1e-5

QC = 256          # query-chunk width
NQC = S // QC     # 8
NKT = S // 128    # 16 key tiles
NG = 4            # score groups per unit (4 ktiles each)

# fp8 range scales (folded into host weight prep)
QKSCALE = 8.0     # q and k each scaled 8x -> fp8-friendly
VSCALE = 16.0     # v scaled 16x (cancelled by RMS)
WOSCALE = 64.0    # Wo scaled 64x, removed in the psum->sbuf copy
ESC = (D ** -0.5) / (QKSCALE * QKSCALE)   # exp scale on raw score psum
LOG2E = 1.0 / math.log(2.0)
A16 = 128.0 * LOG2E * ESC                 # Schraudolph slope (bf16 bits)
B16 = 16251.0                             # tuned offset (max rel err 3.3%)

# exp engine per (unit_index, comp): 'A' ScalarE (fp8 P, DoubleRow PV),
# 'V' DVE, 'P' GpSimd (both bf16 P, plain PV).  64 entries, tuned so all
# three engines finish together.
def _make_assign(na, nv, npp):
    # interleave A/V/P counts evenly over 64 slots (largest remainder)
    total = na + nv + npp
    assert total == 64
    out = []
    cnt = {"A": 0, "V": 0, "P": 0}
    want = {"A": na, "V": nv, "P": npp}
    for i in range(64):
        # pick the engine furthest behind its quota
        best = max("AVP", key=lambda e: want[e] * (i + 1) / 64 - cnt[e])
        out.append(best)
        cnt[best] += 1
    return out


EXP_ASSIGN = [(["V", "A", "A", "A"] if i % 2 == 0 else
               ["V", "A", "V", "A"])[j] for i in range(16) for j in range(4)]


def build_kernel(reps: int = 1):
    nc = bacc.Bacc("TRN2", target_bir_lowering=False, debug=False,
                   num_devices=NCORES)
    xbf = nc.dram_tensor("xbf", [E, S], BF16, kind="ExternalInput")
    wq = nc.dram_tensor("wq", [E, FPC], BF16, kind="ExternalInput")
    wk = nc.dram_tensor("wk", [E, FPC], BF16, kind="ExternalInput")
    wv = nc.dram_tensor("wv", [E, FPC], BF16, kind="ExternalInput")
    wob = nc.dram_tensor("wob", [128, 2 * E], BF16, kind="ExternalInput")
    cf32 = nc.dram_tensor("cf32", [128, 1], F32, kind="ExternalInput")
    idb = nc.dram_tensor("idb", [128, 128], BF16, kind="ExternalInput")
    out = nc.dram_tensor("out", [S, E], F32, kind="ExternalOutput")
    DEBUG = getattr(build_kernel, "debug", False)
    if DEBUG:
        dbg_qt = nc.dram_tensor("dbg_qt", [128, S], FP8, kind="ExternalOutput")
        dbg_kt = nc.dram_tensor("dbg_kt", [128, 2 * S], FP8, kind="ExternalOutput")
        dbg_vt = nc.dram_tensor("dbg_vt", [128, NKT * FPC], FP8, kind="ExternalOutput")
        dbg_pt0 = nc.dram_tensor("dbg_pt0", [128, 4096], mybir.dt.uint8, kind="ExternalOutput")
        dbg_pt1 = nc.dram_tensor("dbg_pt1", [128, 4096], mybir.dt.uint8, kind="ExternalOutput")
        dbg_o = nc.dram_tensor("dbg_o", [128, 512], F32, kind="ExternalOutput")
        dbg_r = nc.dram_tensor("dbg_r", [128, 16], F32, kind="ExternalOutput")
        dbg_ab = nc.dram_tensor("dbg_ab", [128, 256], F32, kind="ExternalOutput")

    with tile.TileContext(nc) as tc, ExitStack() as ctx:
        cpool = ctx.enter_context(tc.tile_pool(name="consts", bufs=1))
        ipool = ctx.enter_context(tc.tile_pool(name="inputs", bufs=1))
        qpool = ctx.enter_context(tc.tile_pool(name="qkv", bufs=1))
        pt16p = ctx.enter_context(tc.tile_pool(name="pt16", bufs=2))
        wpool = ctx.enter_context(tc.tile_pool(name="work", bufs=3))
        pst = ctx.enter_context(tc.tile_pool(name="pst", bufs=2, space="PSUM"))
        po = ctx.enter_context(tc.tile_pool(name="po", bufs=2, space="PSUM"))
        pops = ctx.enter_context(tc.tile_pool(name="pops", bufs=1, space="PSUM"))
        pr = ctx.enter_context(tc.tile_pool(name="pr", bufs=1, space="PSUM"))

        # ---------------- consts ----------------
        lam_sb = cpool.tile([128, 1], F32, tag="lam")
        nc.sync.dma_start(lam_sb[:], cf32.ap())
        idb_sb = cpool.tile([128, 128], BF16, tag="idb")
        nc.sync.dma_start(idb_sb[:], idb.ap())
        eps_sb = cpool.tile([128, 1], F32, tag="eps")
        nc.vector.memset(eps_sb[:], EPS)
        ones_bf = cpool.tile([128, 1], BF16, tag="onesb")
        nc.vector.memset(ones_bf[:], 1.0)

        # ---------------- inputs ----------------
        xbf_sb = ipool.tile([128, 8, S], BF16, tag="xbf")
        wq_sb = ipool.tile([128, 8, FPC], BF16, tag="wq")
        wk_sb = ipool.tile([128, 8, FPC], BF16, tag="wk")
        wv_sb = ipool.tile([128, 8, FPC], BF16, tag="wv")
        wob_sb = ipool.tile([128, 2, E], BF16, tag="wob")
        wkr = wk.ap().rearrange("(kb p) f -> p kb f", p=128)
        nc.sync.dma_start(wk_sb[:, 0:4, :], wkr[:, 0:4, :])
        nc.sync.dma_start(wk_sb[:, 4:8, :], wkr[:, 4:8, :])
        for nch in range(4):
            for kb in range(8):
                eng = (nc.sync, nc.gpsimd)[kb % 2]
                eng.dma_start(
                    xbf_sb[:, kb, nch * 512:(nch + 1) * 512],
                    xbf.ap()[kb * 128:(kb + 1) * 128,
                             nch * 512:(nch + 1) * 512])
            if nch == 0:
                nc.sync.dma_start(
                    wq_sb[:], wq.ap().rearrange("(kb p) f -> p kb f", p=128))
            if nch == 1:
                nc.sync.dma_start(
                    wv_sb[:], wv.ap().rearrange("(kb p) f -> p kb f", p=128))
        nc.sync.dma_start(wob_sb[:], wob.ap())

        # ---------------- persistent qkv tiles ----------------
        # qT/kT: [feat, seq] fp8.  kT has a zero right half: the DoubleRow
        # score matmul pairs [32,2] along free; pair 1 reads zero weights.
        qT = [qpool.tile([128, S], BF16, tag=f"qT{fb}", name="qT")
              for fb in range(2)]
        kT = [qpool.tile([128, S], BF16, tag=f"kT{fb}", name="kT")
              for fb in range(2)]
        vt = qpool.tile([128, NKT * FPC], BF16, tag="vt", name="vt")

        for _rep in range(reps):
            # ------------- projection helpers -------------
            def proj_qk_round(dst, w_sb, fb, nch, copy_eng):
                ps = pops.tile([128, 512], F32, tag="ops", name="ops")
                for kb in range(8):
                    nc.tensor.matmul(
                        ps[:], w_sb[:, kb, fb * 128:(fb + 1) * 128],
                        xbf_sb[:, kb, nch * 512:(nch + 1) * 512],
                        start=(kb == 0), stop=(kb == 7))
                copy_eng.tensor_copy(dst[fb][:, nch * 512:(nch + 1) * 512], ps[:])

            def proj_v_round(st, copy_eng):
                ps = po.tile([128, 512], F32, tag="o", name="vps")
                for kb in range(8):
                    nc.tensor.matmul(
                        ps[:, 0:FPC],
                        xbf_sb[:, kb, st * 128:(st + 1) * 128],
                        wv_sb[:, kb, :],
                        start=(kb == 0), stop=(kb == 7))
                copy_eng.tensor_copy(vt[:, st * FPC:(st + 1) * FPC], ps[:, 0:FPC])

            # ------------- deferred-work scheduler -------------
            # slots at (unit, group) granularity; at(k, fn) runs fn k slots
            # from now.
            sched = deque([[] for _ in range(24)])

            def at(k, fn):
                sched[k].append(fn)

            def pop_slot():
                for fn in sched.popleft():
                    fn()
                sched.append([])

            # prologue: k/q fb0 nch0 only; later chunks land just in
            # time for the score groups that need them.
            proj_qk_round(kT, wk_sb, 0, 0, nc.vector)
            proj_qk_round(qT, wq_sb, 0, 0, nc.vector)

            # deferred projections: (fn, slot) list consumed by early units
            def mk_qk(dst, w_sb, fb, nch, eng):
                return lambda: proj_qk_round(dst, w_sb, fb, nch, eng)

            def mk_v(st, eng):
                return lambda: proj_v_round(st, eng)

            # v rounds during unit 0 (4 per group-slot, ahead of PV use);
            # k fb1 during units 0-1 (needed by unit 2); q rounds spread.
            dwork = {}  # slot index (absolute) -> list of fns

            def dq(slot, fn):
                dwork.setdefault(slot, []).append(fn)

            for st in range(8):
                dq(0, mk_v(st, nc.vector))
            for st in range(8, 16):
                dq(1, mk_v(st, nc.vector))
            dq(0, mk_qk(kT, wk_sb, 0, 1, nc.vector))
            dq(1, mk_qk(kT, wk_sb, 0, 2, nc.vector))
            dq(2, mk_qk(kT, wk_sb, 0, 3, nc.vector))
            for nch in range(4):
                dq(3 + nch, mk_qk(kT, wk_sb, 1, nch, nc.vector))
            dq(7, mk_qk(qT, wq_sb, 1, 0, nc.vector))
            # remaining q chunks: fb0 nch1-3 needed at qc2/4/6 (units 8/16/24
            # -> slots 32/64/96); fb1 similarly.
            dq(12, mk_qk(qT, wq_sb, 0, 1, nc.vector))
            dq(16, mk_qk(qT, wq_sb, 1, 1, nc.vector))
            dq(44, mk_qk(qT, wq_sb, 0, 2, nc.vector))
            dq(48, mk_qk(qT, wq_sb, 1, 2, nc.vector))
            dq(76, mk_qk(qT, wq_sb, 0, 3, nc.vector))
            dq(80, mk_qk(qT, wq_sb, 1, 3, nc.vector))

            # ------------- attention -------------
            units = [(qc, h) for qc in range(NQC) for h in range(HPC)]
            qc_state = {}
            slot_idx = 0

            def fill_half(u, g, c, hb):
                fb, off, qc = u["fb"], u["off"] + 32 * c, u["qc"]
                rhs = qT[fb][off:off + 32, qc * QC:(qc + 1) * QC]
                tp = (off, 0) if off == 96 else None
                st_ps = pst.tile([128, 512], F32, tag=f"st{hb}", name="st")
                for j in range(2):
                    ktile = 4 * g + 2 * hb + j
                    nc.tensor.matmul(
                        st_ps[:, j * QC:(j + 1) * QC],
                        kT[fb][off:off + 32,
                               ktile * 128:(ktile + 1) * 128],
                        rhs, start=True, stop=True, tile_position=tp)
                return st_ps

            def emit_exp(u, g, c, halves):
                eng = u["eng"][c]
                for hb in range(2):
                    sl = u["pt"][c][:, g * 1024 + hb * 512:
                                    g * 1024 + hb * 512 + 512]
                    if eng == "A":
                        nc.scalar.activation(sl, halves[hb][:], ACT.Exp,
                                             scale=ESC)
                    else:
                        nc.vector.tensor_scalar(
                            sl.bitcast(I16), halves[hb][:],
                            A16, B16, op0=ALU.mult, op1=ALU.add)

            def emit_pv(u, g, c):
                qc, h = u["qc"], u["h"]
                o_t = qc_state[qc]["O"]
                ptb = u["pt"][c][:]
                first_chain = (h == 0 and c == 0)
                for qs in range(2):
                    ot_ap = o_t[qs].rearrange("p (h c d) -> p h c d", c=2, d=64)
                    out_ap = ot_ap[:, h, c, :]
                    for jj in range(4):
                        j = 4 * g + jj
                        nc.tensor.matmul(
                            out_ap,
                            ptb[:, j * QC + qs * 128:j * QC + qs * 128 + 128],
                            vt[:, j * FPC + h * 64:j * FPC + h * 64 + 64],
                            start=(g == 0 and jj == 0 and first_chain),
                            stop=(g == NG - 1 and jj == 3),
                            skip_group_check=True)

            def emit_rowsum(u, g, c):
                qc, h = u["qc"], u["h"]
                r_t = qc_state[qc]["r"]
                ptb = u["pt"][c][:]
                for qs in range(2):
                    first_chain = (h == 0 and c == 0 and qs == 0)
                    col = qs * 8 + h * 2 + c
                    out_ap = r_t[:, col:col + 1]
                    for jj in range(4):
                        j = 4 * g + jj
                        nc.tensor.matmul(
                            out_ap,
                            ptb[:, j * QC + qs * 128:j * QC + qs * 128 + 128],
                            ones_bf[:, 0:1],
                            start=(g == 0 and jj == 0 and first_chain),
                            stop=(g == NG - 1 and jj == 3),
                            skip_group_check=True)

            # ------------- per-qc tail -------------
            def mk_normalize(qc, qs):
                def _fn():
                    stt = qc_state[qc]
                    if qs == 0:
                        rall = wpool.tile([128, 16], F32, tag="rall",
                                          name="rall")
                        stt["rall"] = rall
                        nc.vector.tensor_copy(rall[:], stt["r"][:])
                    rall = stt["rall"]
                    rv = rall.rearrange("p (s h c) -> p s h c", s=2, c=2)
                    o_t = stt["O"][qs]
                    ov = o_t.rearrange("p (h c d) -> p h c d", c=2, d=64)
                    t1 = wpool.tile([128, 4, 64], BF16, tag="t1", name="t1")
                    t2 = wpool.tile([128, 4, 64], BF16, tag="t2", name="t2")
                    uu = wpool.tile([128, 4, 64], BF16, tag=f"u{qs}", name="u")
                    s2 = wpool.tile([128, 4, 64], BF16, tag="s2", name="s2")
                    i0, i1 = broadcast_tensor_aps(ov[:, :, 0, :],
                                                  rv[:, qs, :, 1:2])
                    nc.vector.tensor_tensor(t1[:], i0, i1, op=ALU.mult)
                    i0, i1 = broadcast_tensor_aps(ov[:, :, 1, :],
                                                  rv[:, qs, :, 0:1])
                    nc.vector.tensor_tensor(t2[:], i0, i1, op=ALU.mult)
                    # u = lam*t2 - t1  (= -(r2 O1 - lam r1 O2); Wo negated)
                    nc.vector.scalar_tensor_tensor(
                        uu[:], t2[:], lam_sb[:, 0:1], t1[:],
                        op0=ALU.mult, op1=ALU.subtract)
                    nc.gpsimd.tensor_mul(s2[:], uu[:], uu[:])
                    nc.vector.tensor_reduce(
                        stt["ssq"][:, qs, :], s2[:],
                        axis=mybir.AxisListType.X, op=ALU.add)
                    stt[f"u{qs}"] = uu
                return _fn

            def mk_rms(qc, qs=None):
                def _fn():
                    stt = qc_state[qc]
                    if qs is None:
                        rln = wpool.tile([128, 8], F32, tag="rln", name="rln")
                        rmsi = wpool.tile([128, 8], BF16, tag="rmsi",
                                          name="rmsi")
                        nc.scalar.activation(rln[:], stt["ssq"][:].rearrange(
                            "p a b -> p (a b)"), ACT.Ln,
                            scale=1.0 / 64.0, bias=eps_sb[:, 0:1])
                        nc.scalar.activation(rmsi[:], rln[:], ACT.Exp,
                                             scale=-0.5)
                        stt["rmsi"] = rmsi
                        return
                    if "rmsi" not in stt:
                        stt["rmsi"] = wpool.tile([128, 8], BF16, tag="rmsi",
                                                 name="rmsi")
                    rln = wpool.tile([128, 4], F32, tag="rln4", name="rln")
                    nc.scalar.activation(rln[:], stt["ssq"][:, qs, :],
                                         ACT.Ln, scale=1.0 / 64.0,
                                         bias=eps_sb[:, 0:1])
                    nc.scalar.activation(
                        stt["rmsi"][:, qs * 4:(qs + 1) * 4], rln[:],
                        ACT.Exp, scale=-0.5)
                return _fn

            def mk_apply_tp(qc, qs, pool=None, tt_eng=None):
                def _fn():
                    stt = qc_state[qc]
                    uu = stt[f"u{qs}"]
                    rmsi = stt["rmsi"].rearrange("p (s h) -> p s h", s=2)
                    ab = wpool.tile([128, 4, 64], BF16, tag=f"ab{qs}",
                                    name="ab")
                    i0, i1 = broadcast_tensor_aps(
                        uu[:], rmsi[:, qs, :].rearrange("p (h o) -> p h o",
                                                        o=1))
                    (tt_eng or nc.gpsimd).tensor_tensor(ab[:], i0, i1,
                                                       op=ALU.mult)
                    abf = ab.rearrange("p h d -> p (h d)")
                    atp = (pool or pops).tile([128, 512], F32,
                                              tag="ops" if pool is None
                                              else "o", name="atps")
                    atps = atp[:].bitcast(BF16)
                    for fc in range(2):
                        nc.tensor.transpose(
                            atps[:, fc * 136:fc * 136 + 128],
                            abf[:, fc * 128:(fc + 1) * 128], idb_sb[:])
                    atb = wpool.tile([128, 2, 128], BF16, tag=f"at{qs}",
                                     name="atb")
                    tsrc = atps[:, 0:272].rearrange(
                        "p (t f) -> p t f", t=2)[:, :, 0:128]
                    nc.vector.tensor_copy(atb[:], tsrc)
                    stt[f"at{qs}"] = atb
                return _fn

            def mk_oproj(qc, qs, ec, osb_eng, pool=None):
                def _fn():
                    stt = qc_state[qc]
                    atb = stt[f"at{qs}"]
                    ps = (pool or pops).tile([128, 512], F32,
                                             tag="ops" if pool is None
                                             else "o", name="ops")
                    for fc in range(2):
                        nc.tensor.matmul(
                            ps[:], atb[:, fc, :],
                            wob_sb[:, fc, ec * 512:(ec + 1) * 512],
                            start=(fc == 0), stop=(fc == 1))
                    osb = wpool.tile([128, 512], F32, tag="osb", name="osb")
                    if osb_eng is nc.scalar:
                        nc.scalar.copy(osb[:], ps[:])
                    else:
                        osb_eng.tensor_copy(osb[:], ps[:])
                    row = (qc * 2 + qs) * 128
                    nc.sync.dma_start(
                        out.ap()[row:row + 128, ec * 512:(ec + 1) * 512],
                        osb[:])
                return _fn

            def mk_dbg(qc):
                def _fn():
                    stt = qc_state[qc]
                    ou = wpool.tile([128, 512], F32, tag="dbgo", name="dbgo")
                    nc.vector.tensor_copy(ou[:], stt["O"][0][:])
                    nc.sync.dma_start(dbg_o.ap(), ou[:])
                    nc.sync.dma_start(dbg_r.ap(), stt["rall"][:])
                    ab = wpool.tile([128, 256], F32, tag="dbgab", name="dbgab")
                    nc.vector.tensor_copy(
                        ab.rearrange("p (h d) -> p h d", d=64)[:],
                        stt["u0"][:])
                    nc.sync.dma_start(dbg_ab.ap(), ab[:])
                return _fn

            for pi in range(len(units) // 2):
                pair = [units[2 * pi], units[2 * pi + 1]]
                qc = pair[0][0]
                if qc not in qc_state:
                    qc_state[qc] = {
                        "O": [po.tile([128, 512], F32, tag="o",
                                      name=f"O{qs}") for qs in range(2)],
                        "r": None,
                        "ssq": wpool.tile([128, 2, 4], F32, tag="ssq",
                                          name="ssq"),
                    }
                uu = []
                for k, (qc_, h_) in enumerate(pair):
                    ui = 2 * pi + k
                    uu.append({
                        "qc": qc_, "h": h_, "fb": h_ // 2,
                        "off": (h_ % 2) * 64,
                        "eng": (EXP_ASSIGN[2 * ui], EXP_ASSIGN[2 * ui + 1]),
                        "pt": [pt16p.tile([128, 4096], BF16,
                                          tag=f"pt{k}{c}", name="pt16")
                               for c in range(2)],
                    })
                st_cur = {}
                for hb in range(2):
                    for k in range(2):
                        for c in range(2):
                            st_cur.setdefault((k, c), []).append(
                                fill_half(uu[k], 0, c, hb))
                for g in range(NG):
                    if g == 0:
                        pop_slot()  # normalize(prev qc) ahead of exps on DVE
                    for k in range(2):
                        for c in range(2):
                            emit_exp(uu[k], g, c, st_cur[(k, c)])
                    if qc_state[qc]["r"] is None and g >= 1:
                        qc_state[qc]["r"] = pr.tile([128, 16], F32, tag="r",
                                                    name="r")
                    if g >= 1:
                        for k in range(2):
                            for c in range(2):
                                emit_rowsum(uu[k], g - 1, c)
                                emit_pv(uu[k], g - 1, c)
                    for fn in dwork.pop(slot_idx + 2 * g, []):
                        fn()
                    if g > 0:
                        pop_slot()
                    for fn in dwork.pop(slot_idx + 2 * g + 1, []):
                        fn()
                    pop_slot()
                    if g + 1 < NG:
                        st_cur = {}
                        for hb in range(2):
                            for k in range(2):
                                for c in range(2):
                                    st_cur.setdefault((k, c), []).append(
                                        fill_half(uu[k], g + 1, c, hb))
                for k in range(2):
                    for c in range(2):
                        emit_rowsum(uu[k], NG - 1, c)
                        emit_pv(uu[k], NG - 1, c)
                if DEBUG and pi == 7:
                    nc.sync.dma_start(dbg_qt.ap(), qT[0][:])
                    nc.sync.dma_start(dbg_kt.ap(), kT[0][:])
                    nc.sync.dma_start(dbg_vt.ap(), vt[:])
                if DEBUG and qc == 2 and pair[0][1] == 0:
                    for cc, dt_ in ((0, dbg_pt0), (1, dbg_pt1)):
                        tt = uu[0]["pt"][cc]
                        nc.sync.dma_start(
                            dt_.ap()[:, 0:4096],
                            tt[:].bitcast(mybir.dt.uint8)[:, 0:4096])
                if pair[1][1] == HPC - 1:
                    if DEBUG and qc == 2:
                        at(2, mk_dbg(qc))
                    if qc == NQC - 1:
                        at(0, mk_normalize(qc, 0))
                        at(0, mk_rms(qc, 0))
                        at(0, mk_apply_tp(qc, 0, po, nc.vector))
                        at(0, mk_oproj(qc, 0, 0, nc.vector, po))
                        at(0, mk_normalize(qc, 1))
                        at(0, mk_rms(qc, 1))
                        at(0, mk_oproj(qc, 0, 1, nc.scalar, po))
                        at(0, mk_apply_tp(qc, 1, po, nc.gpsimd))
                        at(1, mk_oproj(qc, 1, 0, nc.vector, po))
                        at(1, mk_oproj(qc, 1, 1, nc.scalar, po))
                    else:
                        at(0, mk_normalize(qc, 0))
                        at(1, mk_normalize(qc, 1))
                        at(1, mk_rms(qc))
                        at(2, mk_apply_tp(qc, 0))
                        at(2, mk_oproj(qc, 0, 0, nc.vector))
                        at(3, mk_oproj(qc, 0, 1, nc.vector))
                        at(3, mk_apply_tp(qc, 1))
                        at(4, mk_oproj(qc, 1, 0, nc.vector))
                        at(5, mk_oproj(qc, 1, 1, nc.vector))
                slot_idx += 2 * NG
            # drain remaining scheduled work
            for fns in dwork.values():
                for fn in fns:
                    fn()
            while any(sched):
                pop_slot()
            qc_state.clear()
    nc.compile()
    return nc


def _prep_core_inputs(inputs, core):
    x = np.asarray(inputs["x"], np.float32)
    Wq = np.asarray(inputs["Wq"], np.float32)
    Wk = np.asarray(inputs["Wk"], np.float32)
    Wv = np.asarray(inputs["Wv"], np.float32)
    Wo = np.asarray(inputs["Wo"], np.float32)
    subln_w = np.asarray(inputs["subln_w"], np.float32)
    b, hg = core // 4, core % 4
    sl = slice(FPC * hg, FPC * (hg + 1))
    bf = ml_dtypes.bfloat16
    f8 = ml_dtypes.float8_e4m3
    lam_full = float(
        np.exp(np.sum(np.asarray(inputs["lambda_q1"], np.float64)
                      * np.asarray(inputs["lambda_k1"], np.float64)))
        - np.exp(np.sum(np.asarray(inputs["lambda_q2"], np.float64)
                        * np.asarray(inputs["lambda_k2"], np.float64)))
        + LAMBDA_INIT)
    xT = np.ascontiguousarray(x[b].T)
    wo_scale = (np.tile(subln_w, HPC) * (1.0 - LAMBDA_INIT))
    wo_dev = -(Wo[:, sl].T * wo_scale[:, None])
    wo_dev = np.ascontiguousarray(
        wo_dev.reshape(2, 128, E).transpose(1, 0, 2).reshape(128, 2 * E))
    return {
        "xbf": xT.astype(bf),
        "wq": np.ascontiguousarray(Wq[sl].T * QKSCALE).astype(bf),
        "wk": np.ascontiguousarray(Wk[sl].T * QKSCALE).astype(bf),
        "wv": np.ascontiguousarray(Wv[sl].T).astype(bf),
        "wob": wo_dev.astype(bf),
        "cf32": np.full((128, 1), lam_full, np.float32),
        "idb": np.eye(128, dtype=bf),
    }


_CACHED = {}


def _get_kernel(reps=1):
    if reps not in _CACHED:
        _CACHED[reps] = build_kernel(reps)
    return _CACHED[reps]


def run_on_cores(inputs, reps=1):
    nc = _get_kernel(reps)
    in_maps = [_prep_core_inputs(inputs, c) for c in range(NCORES)]
    res = run_bass_kernel_spmd(nc, in_maps, core_ids=list(range(NCORES)))
    return res


def kernel(**inputs) -> np.ndarray:
    res = run_on_cores(inputs)
    out = np.zeros((B, S, E), np.float32)
    for c in range(NCORES):
        out[c // 4] += res.results[c]["out"]
    return out


# revision 31
# speedup vs baseline: 1.0107x; 1.0027x over previous
"""Differential multi-head attention on 8 Trainium2 NeuronCores.

Sharding: tensor-parallel over heads x data-parallel over batch.
Core c handles batch b = c//4 and real heads [4*(c%4), 4*(c%4)+4).
Each core computes a partial output (its 256 attention features through
the output projection); the host sums the 4 partials per batch.

Per-core design (v2): dual-engine exp + multiplicative RMS normalization.

  The kernel is jointly bound by PE matmuls (~224us bf16 floor) and exp
  over 8 score matrices of [2048, 2048] (262144 activation rows, ~265us
  if one engine did it all).  exp is split between ScalarE (native Exp,
  ~62% of rows) and DVE (1-instruction Schraudolph exp: int16 bits <-
  A*s + B, bitcast to bf16; max rel err 3.3%), per the tuned EXP_ASSIGN
  pattern.  All matmuls are bf16 (fp8/DoubleRow was tried everywhere
  and rejected: e4m3's 3-6% quantization on q/k/v/attn/Wo and its
  240-max range vs scores reaching +-10.4 blow the 2e-2 error budget).

  Normalization exploits RMSNorm scale-invariance to avoid divisions:
  u = r2*O1 - lam*r1*O2 points the same way as O1/r1 - lam*O2/r2 after
  RMS.  Rowsums r come from N=1 matmuls against a ones vector
  (essentially free on PE), lam rides in a fused scalar_tensor_tensor,
  and the per-(q,head) scalars apply via stride-0 broadcast
  tensor_tensor ops batched over all 4 heads.  The square step runs on
  GpSimd (SBUF-only; GPSIMD cannot touch PSUM).  The sign flip is
  folded into a negated Wo on the host; RMS eps is absorbed by u's
  scale; subln_w and (1-lambda_init) fold into Wo.

  Pipelining: units (qc, head) are processed in interleaved PAIRS so
  each exp engine's queue stays back-to-back while the other unit's
  score fills rotate through PSUM.  Score groups use two 1-bank
  [128, 512] half-tiles (tags st0/st1, bufs=2) so a half's fill->exp
  dependency releases independently; PV and rowsum chains lag one
  group as always-ready PE filler.  PSUM: 4 banks scores + 2 banks O
  accumulators (shared with out-proj psum via pool rotation) + 1 bank
  out-proj/transposes + 1 bank rowsums.  The x DMA lands in seq-major
  [128, 512] chunks so the first score group starts after ~1/4 of x;
  weights use single rearranged-AP DMAs.  The final chunk's
  normalize/out-proj chain is special-cased into the freed O banks
  with per-qsub rms to shorten the drain.

Modeled per-core time (TRN2 InstructionCostModel): ~266us
(PE busy ~225us, ScalarE ~200us, DVE ~193us).
"""

import math
import sys

sys.path.insert(0, "/opt/trn_rl_repo")

from collections import deque
from contextlib import ExitStack

import ml_dtypes
import numpy as np

import concourse.bacc as bacc
import concourse.mybir as mybir
import concourse.tile as tile
from concourse.bass import AP, broadcast_tensor_aps
from concourse.bass_utils import run_bass_kernel_spmd

# The kernel's only transcendentals are Exp and Ln; make the activation
# table-set chooser prefer the one set containing both, so a single
# ACT_TABLE_LOAD covers the whole kernel.
_orig_get_activation_tables = bacc.get_activation_tables


def _tables_ln_exp_pinned(arch):
    t = dict(_orig_get_activation_tables(arch))
    pref = "natural_log_exp_and_others"
    if pref not in t:
        return t
    A = mybir.ActivationFunctionType
    out = {}
    for k, v in t.items():
        if k != pref:
            v = {f for f in v if f not in (A.Exp, A.Ln)}
        out[k] = v
    return out


bacc.get_activation_tables = _tables_ln_exp_pinned

F32 = mybir.dt.float32
BF16 = mybir.dt.bfloat16
FP8 = mybir.dt.float8e4
I16 = mybir.dt.int16
ALU = mybir.AluOpType
ACT = mybir.ActivationFunctionType
PM = mybir.MatmulPerfMode

E = 1024          # embed dim
S = 2048          # sequence length
B = 2             # batch
H = 16            # real heads
D = 32            # head dim (per component)
NCORES = 8
HPC = 4           # real heads per core
FPC = HPC * 2 * D  # features per core for q/k/v slices = 256
LAMBDA_INIT = 0.8 - 0.6 * math.exp(-0.3 * 12)
EPS = 1e-5

QC = 256          # query-chunk width
NQC = S // QC     # 8
NKT = S // 128    # 16 key tiles
NG = 4            # score groups per unit (4 ktiles each)

# fp8 range scales (folded into host weight prep)
QKSCALE = 8.0     # q and k each scaled 8x -> fp8-friendly
VSCALE = 16.0     # v scaled 16x (cancelled by RMS)
WOSCALE = 64.0    # Wo scaled 64x, removed in the psum->sbuf copy
ESC = (D ** -0.5) / (QKSCALE * QKSCALE)   # exp scale on raw score psum
LOG2E = 1.0 / math.log(2.0)
A16 = 128.0 * LOG2E * ESC                 # Schraudolph slope (bf16 bits)
B16 = 16251.0                             # tuned offset (max rel err 3.3%)

# exp engine per (unit_index, comp): 'A' ScalarE (fp8 P, DoubleRow PV),
# 'V' DVE, 'P' GpSimd (both bf16 P, plain PV).  64 entries, tuned so all
# three engines finish together.
def _make_assign(na, nv, npp):
    # interleave A/V/P counts evenly over 64 slots (largest remainder)
    total = na + nv + npp
    assert total == 64
    out = []
    cnt = {"A": 0, "V": 0, "P": 0}
    want = {"A": na, "V": nv, "P": npp}
    for i in range(64):
        # pick the engine furthest behind its quota
        best = max("AVP", key=lambda e: want[e] * (i + 1) / 64 - cnt[e])
        out.append(best)
        cnt[best] += 1
    return out


EXP_ASSIGN = [(["V", "A", "A", "A"] if i % 2 == 0 else
               ["V", "A", "V", "A"])[j] for i in range(16) for j in range(4)]


def build_kernel(reps: int = 1):
    nc = bacc.Bacc("TRN2", target_bir_lowering=False, debug=False,
                   num_devices=NCORES)
    xbf = nc.dram_tensor("xbf", [E, S], BF16, kind="ExternalInput")
    wq = nc.dram_tensor("wq", [E, FPC], BF16, kind="ExternalInput")
    wk = nc.dram_tensor("wk", [E, FPC], BF16, kind="ExternalInput")
    wv = nc.dram_tensor("wv", [E, FPC], BF16, kind="ExternalInput")
    wob = nc.dram_tensor("wob", [128, 2 * E], BF16, kind="ExternalInput")
    cf32 = nc.dram_tensor("cf32", [128, 1], F32, kind="ExternalInput")
    idb = nc.dram_tensor("idb", [128, 128], BF16, kind="ExternalInput")
    out = nc.dram_tensor("out", [S, E], F32, kind="ExternalOutput")
    DEBUG = getattr(build_kernel, "debug", False)
    if DEBUG:
        dbg_qt = nc.dram_tensor("dbg_qt", [128, S], FP8, kind="ExternalOutput")
        dbg_kt = nc.dram_tensor("dbg_kt", [128, 2 * S], FP8, kind="ExternalOutput")
        dbg_vt = nc.dram_tensor("dbg_vt", [128, NKT * FPC], FP8, kind="ExternalOutput")
        dbg_pt0 = nc.dram_tensor("dbg_pt0", [128, 4096], mybir.dt.uint8, kind="ExternalOutput")
        dbg_pt1 = nc.dram_tensor("dbg_pt1", [128, 4096], mybir.dt.uint8, kind="ExternalOutput")
        dbg_o = nc.dram_tensor("dbg_o", [128, 512], F32, kind="ExternalOutput")
        dbg_r = nc.dram_tensor("dbg_r", [128, 16], F32, kind="ExternalOutput")
        dbg_ab = nc.dram_tensor("dbg_ab", [128, 256], F32, kind="ExternalOutput")

    with tile.TileContext(nc) as tc, ExitStack() as ctx:
        cpool = ctx.enter_context(tc.tile_pool(name="consts", bufs=1))
        ipool = ctx.enter_context(tc.tile_pool(name="inputs", bufs=1))
        qpool = ctx.enter_context(tc.tile_pool(name="qkv", bufs=1))
        pt16p = ctx.enter_context(tc.tile_pool(name="pt16", bufs=3))
        wpool = ctx.enter_context(tc.tile_pool(name="work", bufs=4))
        pst = ctx.enter_context(tc.tile_pool(name="pst", bufs=2, space="PSUM"))
        po = ctx.enter_context(tc.tile_pool(name="po", bufs=2, space="PSUM"))
        pops = ctx.enter_context(tc.tile_pool(name="pops", bufs=1, space="PSUM"))
        pr = ctx.enter_context(tc.tile_pool(name="pr", bufs=1, space="PSUM"))

        # ---------------- consts ----------------
        lam_sb = cpool.tile([128, 1], F32, tag="lam")
        nc.sync.dma_start(lam_sb[:], cf32.ap())
        idb_sb = cpool.tile([128, 128], BF16, tag="idb")
        nc.sync.dma_start(idb_sb[:], idb.ap())
        eps_sb = cpool.tile([128, 1], F32, tag="eps")
        nc.vector.memset(eps_sb[:], EPS)
        ones_bf = cpool.tile([128, 1], BF16, tag="onesb")
        nc.vector.memset(ones_bf[:], 1.0)

        # ---------------- inputs ----------------
        xbf_sb = ipool.tile([128, 8, S], BF16, tag="xbf")
        wq_sb = ipool.tile([128, 8, FPC], BF16, tag="wq")
        wk_sb = ipool.tile([128, 8, FPC], BF16, tag="wk")
        wv_sb = ipool.tile([128, 8, FPC], BF16, tag="wv")
        wob_sb = ipool.tile([128, 2, E], BF16, tag="wob")
        wkr = wk.ap().rearrange("(kb p) f -> p kb f", p=128)
        nc.sync.dma_start(wk_sb[:, 0:4, :], wkr[:, 0:4, :])
        nc.sync.dma_start(wk_sb[:, 4:8, :], wkr[:, 4:8, :])
        for nch in range(4):
            for kb in range(8):
                eng = (nc.sync, nc.gpsimd)[kb % 2]
                eng.dma_start(
                    xbf_sb[:, kb, nch * 512:(nch + 1) * 512],
                    xbf.ap()[kb * 128:(kb + 1) * 128,
                             nch * 512:(nch + 1) * 512])
            if nch == 0:
                nc.sync.dma_start(
                    wq_sb[:], wq.ap().rearrange("(kb p) f -> p kb f", p=128))
            if nch == 1:
                nc.sync.dma_start(
                    wv_sb[:], wv.ap().rearrange("(kb p) f -> p kb f", p=128))
        nc.sync.dma_start(wob_sb[:], wob.ap())

        # ---------------- persistent qkv tiles ----------------
        # qT/kT: [feat, seq] fp8.  kT has a zero right half: the DoubleRow
        # score matmul pairs [32,2] along free; pair 1 reads zero weights.
        qT = [qpool.tile([128, S], BF16, tag=f"qT{fb}", name="qT")
              for fb in range(2)]
        kT = [qpool.tile([128, S], BF16, tag=f"kT{fb}", name="kT")
              for fb in range(2)]
        vt = qpool.tile([128, NKT * FPC], BF16, tag="vt", name="vt")

        for _rep in range(reps):
            # ------------- projection helpers -------------
            def proj_qk_round(dst, w_sb, fb, nch, copy_eng):
                ps = pops.tile([128, 512], F32, tag="ops", name="ops")
                for kb in range(8):
                    nc.tensor.matmul(
                        ps[:], w_sb[:, kb, fb * 128:(fb + 1) * 128],
                        xbf_sb[:, kb, nch * 512:(nch + 1) * 512],
                        start=(kb == 0), stop=(kb == 7))
                copy_eng.tensor_copy(dst[fb][:, nch * 512:(nch + 1) * 512], ps[:])

            def proj_v_round(st, copy_eng):
                ps = po.tile([128, 512], F32, tag="o", name="vps")
                for kb in range(8):
                    nc.tensor.matmul(
                        ps[:, 0:FPC],
                        xbf_sb[:, kb, st * 128:(st + 1) * 128],
                        wv_sb[:, kb, :],
                        start=(kb == 0), stop=(kb == 7))
                copy_eng.tensor_copy(vt[:, st * FPC:(st + 1) * FPC], ps[:, 0:FPC])

            # ------------- deferred-work scheduler -------------
            # slots at (unit, group) granularity; at(k, fn) runs fn k slots
            # from now.
            sched = deque([[] for _ in range(24)])

            def at(k, fn):
                sched[k].append(fn)

            def pop_slot():
                for fn in sched.popleft():
                    fn()
                sched.append([])

            # prologue: k/q fb0 nch0 only; later chunks land just in
            # time for the score groups that need them.
            proj_qk_round(kT, wk_sb, 0, 0, nc.vector)
            proj_qk_round(qT, wq_sb, 0, 0, nc.vector)

            # deferred projections: (fn, slot) list consumed by early units
            def mk_qk(dst, w_sb, fb, nch, eng):
                return lambda: proj_qk_round(dst, w_sb, fb, nch, eng)

            def mk_v(st, eng):
                return lambda: proj_v_round(st, eng)

            # v rounds during unit 0 (4 per group-slot, ahead of PV use);
            # k fb1 during units 0-1 (needed by unit 2); q rounds spread.
            dwork = {}  # slot index (absolute) -> list of fns

            def dq(slot, fn):
                dwork.setdefault(slot, []).append(fn)

            for st in range(8):
                dq(0, mk_v(st, nc.vector))
            for st in range(8, 16):
                dq(1, mk_v(st, nc.vector))
            dq(0, mk_qk(kT, wk_sb, 0, 1, nc.vector))
            dq(1, mk_qk(kT, wk_sb, 0, 2, nc.vector))
            dq(2, mk_qk(kT, wk_sb, 0, 3, nc.vector))
            for nch in range(4):
                dq(3 + nch, mk_qk(kT, wk_sb, 1, nch, nc.vector))
            dq(7, mk_qk(qT, wq_sb, 1, 0, nc.vector))
            # remaining q chunks: fb0 nch1-3 needed at qc2/4/6 (units 8/16/24
            # -> slots 32/64/96); fb1 similarly.
            dq(12, mk_qk(qT, wq_sb, 0, 1, nc.vector))
            dq(16, mk_qk(qT, wq_sb, 1, 1, nc.vector))
            dq(44, mk_qk(qT, wq_sb, 0, 2, nc.vector))
            dq(48, mk_qk(qT, wq_sb, 1, 2, nc.vector))
            dq(76, mk_qk(qT, wq_sb, 0, 3, nc.vector))
            dq(80, mk_qk(qT, wq_sb, 1, 3, nc.vector))

            # ------------- attention -------------
            units = [(qc, h) for qc in range(NQC) for h in range(HPC)]
            qc_state = {}
            slot_idx = 0

            def fill_half(u, g, c, hb):
                fb, off, qc = u["fb"], u["off"] + 32 * c, u["qc"]
                rhs = qT[fb][off:off + 32, qc * QC:(qc + 1) * QC]
                tp = (off, 0) if off == 96 else None
                st_ps = pst.tile([128, 512], F32, tag=f"st{hb}", name="st")
                for j in range(2):
                    ktile = 4 * g + 2 * hb + j
                    nc.tensor.matmul(
                        st_ps[:, j * QC:(j + 1) * QC],
                        kT[fb][off:off + 32,
                               ktile * 128:(ktile + 1) * 128],
                        rhs, start=True, stop=True, tile_position=tp)
                return st_ps

            def emit_exp(u, g, c, halves):
                eng = u["eng"][c]
                for hb in range(2):
                    sl = u["pt"][c][:, g * 1024 + hb * 512:
                                    g * 1024 + hb * 512 + 512]
                    if eng == "A":
                        nc.scalar.activation(sl, halves[hb][:], ACT.Exp,
                                             scale=ESC)
                    else:
                        nc.vector.tensor_scalar(
                            sl.bitcast(I16), halves[hb][:],
                            A16, B16, op0=ALU.mult, op1=ALU.add)

            def emit_pv(u, g, c):
                qc, h = u["qc"], u["h"]
                o_t = qc_state[qc]["O"]
                ptb = u["pt"][c][:]
                first_chain = (h == 0 and c == 0)
                for qs in range(2):
                    ot_ap = o_t[qs].rearrange("p (h c d) -> p h c d", c=2, d=64)
                    out_ap = ot_ap[:, h, c, :]
                    for jj in range(4):
                        j = 4 * g + jj
                        nc.tensor.matmul(
                            out_ap,
                            ptb[:, j * QC + qs * 128:j * QC + qs * 128 + 128],
                            vt[:, j * FPC + h * 64:j * FPC + h * 64 + 64],
                            start=(g == 0 and jj == 0 and first_chain),
                            stop=(g == NG - 1 and jj == 3),
                            skip_group_check=True)

            def emit_rowsum(u, g, c):
                qc, h = u["qc"], u["h"]
                r_t = qc_state[qc]["r"]
                ptb = u["pt"][c][:]
                for qs in range(2):
                    first_chain = (h == 0 and c == 0 and qs == 0)
                    col = qs * 8 + h * 2 + c
                    out_ap = r_t[:, col:col + 1]
                    for jj in range(4):
                        j = 4 * g + jj
                        nc.tensor.matmul(
                            out_ap,
                            ptb[:, j * QC + qs * 128:j * QC + qs * 128 + 128],
                            ones_bf[:, 0:1],
                            start=(g == 0 and jj == 0 and first_chain),
                            stop=(g == NG - 1 and jj == 3),
                            skip_group_check=True)

            # ------------- per-qc tail -------------
            def mk_normalize(qc, qs):
                def _fn():
                    stt = qc_state[qc]
                    if qs == 0:
                        rall = wpool.tile([128, 16], F32, tag="rall",
                                          name="rall")
                        stt["rall"] = rall
                        nc.vector.tensor_copy(rall[:], stt["r"][:])
                    rall = stt["rall"]
                    rv = rall.rearrange("p (s h c) -> p s h c", s=2, c=2)
                    o_t = stt["O"][qs]
                    ov = o_t.rearrange("p (h c d) -> p h c d", c=2, d=64)
                    t1 = wpool.tile([128, 4, 64], BF16, tag="t1", name="t1")
                    t2 = wpool.tile([128, 4, 64], BF16, tag="t2", name="t2")
                    uu = wpool.tile([128, 4, 64], BF16, tag=f"u{qs}", name="u")
                    s2 = wpool.tile([128, 4, 64], BF16, tag="s2", name="s2")
                    i0, i1 = broadcast_tensor_aps(ov[:, :, 0, :],
                                                  rv[:, qs, :, 1:2])
                    nc.vector.tensor_tensor(t1[:], i0, i1, op=ALU.mult)
                    i0, i1 = broadcast_tensor_aps(ov[:, :, 1, :],
                                                  rv[:, qs, :, 0:1])
                    nc.vector.tensor_tensor(t2[:], i0, i1, op=ALU.mult)
                    # u = lam*t2 - t1  (= -(r2 O1 - lam r1 O2); Wo negated)
                    nc.vector.scalar_tensor_tensor(
                        uu[:], t2[:], lam_sb[:, 0:1], t1[:],
                        op0=ALU.mult, op1=ALU.subtract)
                    nc.gpsimd.tensor_mul(s2[:], uu[:], uu[:])
                    nc.vector.tensor_reduce(
                        stt["ssq"][:, qs, :], s2[:],
                        axis=mybir.AxisListType.X, op=ALU.add)
                    stt[f"u{qs}"] = uu
                return _fn

            def mk_rms(qc, qs=None):
                def _fn():
                    stt = qc_state[qc]
                    if qs is None:
                        rln = wpool.tile([128, 8], F32, tag="rln", name="rln")
                        rmsi = wpool.tile([128, 8], BF16, tag="rmsi",
                                          name="rmsi")
                        nc.scalar.activation(rln[:], stt["ssq"][:].rearrange(
                            "p a b -> p (a b)"), ACT.Ln,
                            scale=1.0 / 64.0, bias=eps_sb[:, 0:1])
                        nc.scalar.activation(rmsi[:], rln[:], ACT.Exp,
                                             scale=-0.5)
                        stt["rmsi"] = rmsi
                        return
                    if "rmsi" not in stt:
                        stt["rmsi"] = wpool.tile([128, 8], BF16, tag="rmsi",
                                                 name="rmsi")
                    rln = wpool.tile([128, 4], F32, tag="rln4", name="rln")
                    nc.scalar.activation(rln[:], stt["ssq"][:, qs, :],
                                         ACT.Ln, scale=1.0 / 64.0,
                                         bias=eps_sb[:, 0:1])
                    nc.scalar.activation(
                        stt["rmsi"][:, qs * 4:(qs + 1) * 4], rln[:],
                        ACT.Exp, scale=-0.5)
                return _fn

            def mk_apply_tp(qc, qs, pool=None, tt_eng=None):
                def _fn():
                    stt = qc_state[qc]
                    uu = stt[f"u{qs}"]
                    rmsi = stt["rmsi"].rearrange("p (s h) -> p s h", s=2)
                    ab = wpool.tile([128, 4, 64], BF16, tag=f"ab{qs}",
                                    name="ab")
                    i0, i1 = broadcast_tensor_aps(
                        uu[:], rmsi[:, qs, :].rearrange("p (h o) -> p h o",
                                                        o=1))
                    (tt_eng or nc.gpsimd).tensor_tensor(ab[:], i0, i1,
                                                       op=ALU.mult)
                    abf = ab.rearrange("p h d -> p (h d)")
                    atp = (pool or pops).tile([128, 512], F32,
                                              tag="ops" if pool is None
                                              else "o", name="atps")
                    atps = atp[:].bitcast(BF16)
                    for fc in range(2):
                        nc.tensor.transpose(
                            atps[:, fc * 136:fc * 136 + 128],
                            abf[:, fc * 128:(fc + 1) * 128], idb_sb[:])
                    atb = wpool.tile([128, 2, 128], BF16, tag=f"at{qs}",
                                     name="atb")
                    tsrc = atps[:, 0:272].rearrange(
                        "p (t f) -> p t f", t=2)[:, :, 0:128]
                    nc.vector.tensor_copy(atb[:], tsrc)
                    stt[f"at{qs}"] = atb
                return _fn

            def mk_oproj(qc, qs, ec, osb_eng, pool=None):
                def _fn():
                    stt = qc_state[qc]
                    atb = stt[f"at{qs}"]
                    ps = (pool or pops).tile([128, 512], F32,
                                             tag="ops" if pool is None
                                             else "o", name="ops")
                    for fc in range(2):
                        nc.tensor.matmul(
                            ps[:], atb[:, fc, :],
                            wob_sb[:, fc, ec * 512:(ec + 1) * 512],
                            start=(fc == 0), stop=(fc == 1))
                    osb = wpool.tile([128, 512], F32, tag="osb", name="osb")
                    if osb_eng is nc.scalar:
                        nc.scalar.copy(osb[:], ps[:])
                    else:
                        osb_eng.tensor_copy(osb[:], ps[:])
                    row = (qc * 2 + qs) * 128
                    nc.sync.dma_start(
                        out.ap()[row:row + 128, ec * 512:(ec + 1) * 512],
                        osb[:])
                return _fn

            def mk_dbg(qc):
                def _fn():
                    stt = qc_state[qc]
                    ou = wpool.tile([128, 512], F32, tag="dbgo", name="dbgo")
                    nc.vector.tensor_copy(ou[:], stt["O"][0][:])
                    nc.sync.dma_start(dbg_o.ap(), ou[:])
                    nc.sync.dma_start(dbg_r.ap(), stt["rall"][:])
                    ab = wpool.tile([128, 256], F32, tag="dbgab", name="dbgab")
                    nc.vector.tensor_copy(
                        ab.rearrange("p (h d) -> p h d", d=64)[:],
                        stt["u0"][:])
                    nc.sync.dma_start(dbg_ab.ap(), ab[:])
                return _fn

            for pi in range(len(units) // 2):
                pair = [units[2 * pi], units[2 * pi + 1]]
                qc = pair[0][0]
                if qc not in qc_state:
                    qc_state[qc] = {
                        "O": [po.tile([128, 512], F32, tag="o",
                                      name=f"O{qs}") for qs in range(2)],
                        "r": None,
                        "ssq": wpool.tile([128, 2, 4], F32, tag="ssq",
                                          name="ssq"),
                    }
                uu = []
                for k, (qc_, h_) in enumerate(pair):
                    ui = 2 * pi + k
                    uu.append({
                        "qc": qc_, "h": h_, "fb": h_ // 2,
                        "off": (h_ % 2) * 64,
                        "eng": (EXP_ASSIGN[2 * ui], EXP_ASSIGN[2 * ui + 1]),
                        "pt": [pt16p.tile([128, 4096], BF16,
                                          tag=f"pt{k}{c}", name="pt16")
                               for c in range(2)],
                    })
                st_cur = {}
                for hb in range(2):
                    for k in range(2):
                        for c in range(2):
                            st_cur.setdefault((k, c), []).append(
                                fill_half(uu[k], 0, c, hb))
                for g in range(NG):
                    if g == 0:
                        pop_slot()  # normalize(prev qc) ahead of exps on DVE
                    for k in range(2):
                        for c in range(2):
                            emit_exp(uu[k], g, c, st_cur[(k, c)])
                    if qc_state[qc]["r"] is None and g >= 1:
                        qc_state[qc]["r"] = pr.tile([128, 16], F32, tag="r",
                                                    name="r")
                    if g >= 1:
                        for k in range(2):
                            for c in range(2):
                                emit_rowsum(uu[k], g - 1, c)
                                emit_pv(uu[k], g - 1, c)
                    for fn in dwork.pop(slot_idx + 2 * g, []):
                        fn()
                    if g > 0:
                        pop_slot()
                    for fn in dwork.pop(slot_idx + 2 * g + 1, []):
                        fn()
                    pop_slot()
                    if g + 1 < NG:
                        st_cur = {}
                        for hb in range(2):
                            for k in range(2):
                                for c in range(2):
                                    st_cur.setdefault((k, c), []).append(
                                        fill_half(uu[k], g + 1, c, hb))
                for k in range(2):
                    for c in range(2):
                        emit_rowsum(uu[k], NG - 1, c)
                        emit_pv(uu[k], NG - 1, c)
                if DEBUG and pi == 7:
                    nc.sync.dma_start(dbg_qt.ap(), qT[0][:])
                    nc.sync.dma_start(dbg_kt.ap(), kT[0][:])
                    nc.sync.dma_start(dbg_vt.ap(), vt[:])
                if DEBUG and qc == 2 and pair[0][1] == 0:
                    for cc, dt_ in ((0, dbg_pt0), (1, dbg_pt1)):
                        tt = uu[0]["pt"][cc]
                        nc.sync.dma_start(
                            dt_.ap()[:, 0:4096],
                            tt[:].bitcast(mybir.dt.uint8)[:, 0:4096])
                if pair[1][1] == HPC - 1:
                    if DEBUG and qc == 2:
                        at(2, mk_dbg(qc))
                    if qc == NQC - 1:
                        at(0, mk_normalize(qc, 0))
                        at(0, mk_rms(qc, 0))
                        at(0, mk_apply_tp(qc, 0, po, nc.vector))
                        at(0, mk_oproj(qc, 0, 0, nc.vector, po))
                        at(0, mk_normalize(qc, 1))
                        at(0, mk_rms(qc, 1))
                        at(0, mk_oproj(qc, 0, 1, nc.scalar, po))
                        at(0, mk_apply_tp(qc, 1, po, nc.gpsimd))
                        at(1, mk_oproj(qc, 1, 0, nc.vector, po))
                        at(1, mk_oproj(qc, 1, 1, nc.scalar, po))
                    else:
                        at(0, mk_normalize(qc, 0))
                        at(1, mk_normalize(qc, 1))
                        at(1, mk_rms(qc))
                        at(2, mk_apply_tp(qc, 0))
                        at(2, mk_oproj(qc, 0, 0, nc.vector))
                        at(3, mk_oproj(qc, 0, 1, nc.vector))
                        at(3, mk_apply_tp(qc, 1))
                        at(4, mk_oproj(qc, 1, 0, nc.vector))
                        at(5, mk_oproj(qc, 1, 1, nc.vector))
                slot_idx += 2 * NG
            # drain remaining scheduled work
            for fns in dwork.values():
                for fn in fns:
                    fn()
            while any(sched):
                pop_slot()
            qc_state.clear()
    nc.compile()
    return nc


def _prep_core_inputs(inputs, core):
    x = np.asarray(inputs["x"], np.float32)
    Wq = np.asarray(inputs["Wq"], np.float32)
    Wk = np.asarray(inputs["Wk"], np.float32)
    Wv = np.asarray(inputs["Wv"], np.float32)
    Wo = np.asarray(inputs["Wo"], np.float32)
    subln_w = np.asarray(inputs["subln_w"], np.float32)
    b, hg = core // 4, core % 4
    sl = slice(FPC * hg, FPC * (hg + 1))
    bf = ml_dtypes.bfloat16
    f8 = ml_dtypes.float8_e4m3
    lam_full = float(
        np.exp(np.sum(np.asarray(inputs["lambda_q1"], np.float64)
                      * np.asarray(inputs["lambda_k1"], np.float64)))
        - np.exp(np.sum(np.asarray(inputs["lambda_q2"], np.float64)
                        * np.asarray(inputs["lambda_k2"], np.float64)))
        + LAMBDA_INIT)
    xT = np.ascontiguousarray(x[b].T)
    wo_scale = (np.tile(subln_w, HPC) * (1.0 - LAMBDA_INIT))
    wo_dev = -(Wo[:, sl].T * wo_scale[:, None])
    wo_dev = np.ascontiguousarray(
        wo_dev.reshape(2, 128, E).transpose(1, 0, 2).reshape(128, 2 * E))
    return {
        "xbf": xT.astype(bf),
        "wq": np.ascontiguousarray(Wq[sl].T * QKSCALE).astype(bf),
        "wk": np.ascontiguousarray(Wk[sl].T * QKSCALE).astype(bf),
        "wv": np.ascontiguousarray(Wv[sl].T).astype(bf),
        "wob": wo_dev.astype(bf),
        "cf32": np.full((128, 1), lam_full, np.float32),
        "idb": np.eye(128, dtype=bf),
    }


_CACHED = {}


def _get_kernel(reps=1):
    if reps not in _CACHED:
        _CACHED[reps] = build_kernel(reps)
    return _CACHED[reps]


def run_on_cores(inputs, reps=1):
    nc = _get_kernel(reps)
    in_maps = [_prep_core_inputs(inputs, c) for c in range(NCORES)]
    res = run_bass_kernel_spmd(nc, in_maps, core_ids=list(range(NCORES)))
    return res


def kernel(**inputs) -> np.ndarray:
    res = run_on_cores(inputs)
    out = np.zeros((B, S, E), np.float32)
    for c in range(NCORES):
        out[c // 4] += res.results[c]["out"]
    return out
